# revision 13
# baseline (speedup 1.0000x reference)
"""HGT-style heterogeneous graph message passing on 8 Trainium2 cores.

v3: Pool-descriptor-minimized variant.  v2's wall was the GpSimd (Pool)
engine serially generating SWDGE descriptors at ~8ns/row for ~162k
gathered rows per core (pair-table x gather + own-node x gather +
per-edge-slot table gather).  v3 ships the x rows that the table build
and Q phase need ALREADY GATHERED AND TRANSPOSED from the host (the
sharding hint's "each device holds its edge slice + gathered src/dst
features"), so the only SWDGE gather left is the irreducible per-edge-slot
fetch of the on-device-computed [k'|m] pair rows (~83k rows/core).

 - Host folds the per-(head, etype) relation transforms into per-(ntype,
   etype) 64x128 weight matrices:  a_e = <k'_src, q_dst>  with
   k' = x @ Wk[nt] @ blockdiag_h(A A^T * pri / sqrt(d)),  m = x @ Wv[nt] @
   blockdiag_h(M).
 - dst nodes are sharded across the 8 cores round-robin by degree rank, so
   all segment ops (softmax max/sum, weighted aggregation) become dense row
   reductions over degree-sorted [128, W_t] tiles.  No collectives.
 - Each core builds its deduplicated (src, etype) pair table [rows, 128] =
   [k' | m] in bf16 with dense matmuls over host-pregathered xpairT slabs
   (no on-device gather, no PE transposes), writes it to DRAM partition-major
   (contiguous HWDGE descriptors), then dma_gather's its edge slots
   (256B/row) — the one remaining Pool-engine cost.
"""

import sys

sys.path.insert(0, "/opt/trn_rl_repo")

import numpy as np
import ml_dtypes

BF16 = ml_dtypes.bfloat16

N, E = 40000, 640000
IN, H, HS = 64, 4, 16
NT, ET = 4, 8
D = H * HS  # 64
C = 8  # cores
NL = 5120  # padded local nodes per core
NTILES = NL // 128  # 40
NEG = -1.0e30

_cache = {}


def _host_prep(x, ntype, etype, src, dst):
    """Returns per-core input arrays + structural constants."""
    x = np.ascontiguousarray(np.asarray(x, dtype=np.float32))
    nt_ = np.asarray(ntype).astype(np.int64)
    et_ = np.asarray(etype).astype(np.int64)
    src = np.asarray(src).astype(np.int64)
    dst = np.asarray(dst).astype(np.int64)

    deg = np.bincount(dst, minlength=N)
    order = np.argsort(-deg, kind="stable")
    ranks = np.empty(N, dtype=np.int64)
    ranks[order] = np.arange(N)
    core_of_node = ranks % C
    local_of_node = ranks // C

    # tile widths (shared across cores): tile t covers global ranks [1024t, 1024(t+1))
    W = np.zeros(NTILES, dtype=np.int64)
    deg_by_rank = deg[order]
    for t in range(NTILES):
        lo, hi = t * 1024, min((t + 1) * 1024, N)
        W[t] = max(int(deg_by_rank[lo:hi].max()) if hi > lo else 1, 1)

    percore = []
    for c in range(C):
        ei = np.nonzero(core_of_node[dst] == c)[0]
        ld = local_of_node[dst[ei]]
        o = np.argsort(ld, kind="stable")
        percore.append((ei[o], ld[o]))

    # table chunks: small leading chunks so the first gathers start early;
    # each chunk's padded pair count must stay < 32000 (int16 gather idxs)
    CB = [0, 1, 2, 4, 7, 11, 16, 22, 28, 34, NTILES]
    NCH = len(CB) - 1
    chunk_of_tile = np.zeros(NTILES, dtype=np.int64)
    for h in range(NCH):
        chunk_of_tile[CB[h] : CB[h + 1]] = h
    cnts = np.zeros((C, NCH, NT * ET), dtype=np.int64)
    pair_data = []
    for c in range(C):
        ei, ld = percore[c]
        ch_of = chunk_of_tile[ld // 128]
        key = src[ei] * ET + et_[ei]
        chunk_pairs = []
        for h in range(NCH):
            uk = np.unique(key[ch_of == h])  # sorted keys
            g = nt_[uk // ET] * ET + (uk % ET)
            np.add.at(cnts[c, h], g, 1)
            chunk_pairs.append((uk, g))
        pair_data.append(chunk_pairs)
    # 64-row group alignment: matmul output base partition must be 0/64
    R = 64 * ((cnts.max(axis=0) + 63) // 64)  # [NCH, 32]
    CHRs = 128 * ((R.sum(axis=1) + 127) // 128)
    assert CHRs.max() < 32000, CHRs

    gbase = np.zeros((NCH, NT * ET), dtype=np.int64)
    for h in range(NCH):
        gbase[h] = np.concatenate(([0], np.cumsum(R[h])[:-1]))
    CHRs = [int(v) for v in CHRs]
    RB = np.concatenate(([0], np.cumsum(CHRs)[:-1])).astype(np.int64)
    RPtot = int(sum(CHRs))
    NS = [v // 128 for v in CHRs]  # p-major columns per chunk

    IDX8 = (8 * W).astype(np.int64)
    off8 = np.concatenate(([0], np.cumsum(IDX8)[:-1]))
    offw = np.concatenate(([0], np.cumsum(W)[:-1]))
    IDXW = int(IDX8.sum())
    ABW = int(W.sum())

    cores = []
    own_nodes = np.full((C, NL), -1, dtype=np.int64)
    for c in range(C):
        ei, ld = percore[c]
        etile = ld // 128
        key = src[ei] * ET + et_[ei]

        ownc = order[c::C]
        own_nodes[c, : len(ownc)] = ownc

        ch_of = chunk_of_tile[etile]

        rowid_of_edge = np.zeros(len(ei), dtype=np.int64)
        xp_node = np.full(RPtot, -1, dtype=np.int64)
        for h in range(NCH):
            uk, g = pair_data[c][h]  # uk sorted by key; g aligned
            po = np.argsort(g, kind="stable")
            gs = g[po]
            base_in_g = np.concatenate(
                ([0], np.cumsum(np.bincount(gs, minlength=NT * ET))[:-1])
            )
            rows_po = gbase[h][gs] + (np.arange(len(uk)) - base_in_g[gs])
            row_of_uk = np.empty(len(uk), dtype=np.int64)
            row_of_uk[po] = rows_po
            xp_node[RB[h] + row_of_uk] = uk // ET
            sel = np.nonzero(ch_of == h)[0]
            r = row_of_uk[np.searchsorted(uk, key[sel])]
            # p-major remap: DRAM row' = (r % 128) * NS_h + r // 128
            rowid_of_edge[sel] = (r % 128) * NS[h] + r // 128

        # host-pregathered, transposed x rows for the pair table build
        pn = np.where(xp_node >= 0, xp_node, 0)
        xpairT = np.ascontiguousarray(x[pn].astype(BF16).T)  # [64, RPtot]
        on = np.where(own_nodes[c] >= 0, own_nodes[c], 0)
        xownT = np.ascontiguousarray(x[on].astype(BF16).T)  # [64, NL]

        cnt = np.bincount(ld, minlength=NL)
        starts = np.concatenate(([0], np.cumsum(cnt)[:-1]))
        jpos = np.arange(len(ei)) - starts[ld]
        p_of = ld % 128

        kmidx = np.zeros((16, IDXW), dtype=np.int16)
        for t in range(NTILES):
            wt = int(W[t])
            sel = np.nonzero(etile == t)[0]
            M = np.zeros((128, wt), dtype=np.int16)
            M[p_of[sel], jpos[sel]] = rowid_of_edge[sel].astype(np.int16)
            idsl = M.T.ravel()  # list position k = j*128 + p
            kmidx[:, int(off8[t]) : int(off8[t]) + 8 * wt] = idsl.reshape(
                8 * wt, 16
            ).T

        # abias/oneh are built on device from per-node degree and type
        degf = np.ascontiguousarray(
            cnt.astype(np.float32).reshape(NTILES, 128).T
        )  # [128, NTILES]
        ntv = np.where(own_nodes[c] >= 0, nt_[on], 255).astype(np.float32)
        ntc = np.ascontiguousarray(ntv.reshape(NTILES, 128).T)  # [128, NTILES]

        cores.append(
            dict(
                xpairT=xpairT, xownT=xownT,
                degf=degf, ntc=ntc, kmidx=kmidx,
                iotaf=np.tile(np.arange(int(W.max()), dtype=np.float32), (128, 1)),
            )
        )

    consts = dict(
        W=W, WMAX=int(W.max()), NCH=NCH, CB=CB, R=R, gbase=gbase, CHRs=CHRs,
        NS=NS, RB=RB, RPtot=RPtot, IDXW=IDXW, ABW=ABW, off8=off8, offw=offw,
        own_nodes=own_nodes, deg=deg,
    )
    return cores, consts


def _fold_weights(Wk, Wq, Wv, Wa, rel_att, rel_msg, rel_pri):
    Wk = np.asarray(Wk, np.float64)
    Wq = np.asarray(Wq, np.float64)
    Wv = np.asarray(Wv, np.float64)
    Wa = np.asarray(Wa, np.float64)
    rel_att = np.asarray(rel_att, np.float64)
    rel_msg = np.asarray(rel_msg, np.float64)
    rel_pri = np.asarray(rel_pri, np.float64)
    sd = float(np.sqrt(np.float32(HS)))

    wkm = np.zeros((IN, NT * ET, 2, D), np.float64)
    for nt in range(NT):
        for et in range(ET):
            Batt = np.zeros((D, D))
            Bmsg = np.zeros((D, D))
            for h in range(H):
                A = rel_att[h, et]
                Batt[h * HS : (h + 1) * HS, h * HS : (h + 1) * HS] = (
                    A @ A.T * rel_pri[h, et] / sd
                )
                Bmsg[h * HS : (h + 1) * HS, h * HS : (h + 1) * HS] = rel_msg[h, et]
            g = nt * ET + et
            wkm[:, g, 0] = Wk[nt] @ Batt
            wkm[:, g, 1] = Wv[nt] @ Bmsg
    wkm = wkm.reshape(IN, NT * ET * 2 * D).astype(BF16)
    wkm = np.ascontiguousarray(wkm)  # [64, 4096]
    wqa = np.concatenate([Wq[t] for t in range(NT)], axis=1).astype(BF16)  # [64, 256]
    wa_all = np.concatenate([Wa[t] for t in range(NT)], axis=1).astype(BF16)
    return wkm, wqa, wa_all


def _build_program(consts):
    import concourse.mybir as mybir
    import concourse.tile as tile
    from concourse import bacc
    from concourse.masks import make_identity

    f32 = mybir.dt.float32
    bf16 = mybir.dt.bfloat16
    i16 = mybir.dt.int16
    W = consts["W"]
    WMAX = consts["WMAX"]
    NCH, CB = consts["NCH"], consts["CB"]
    R, gbase, CHRs, RB = consts["R"], consts["gbase"], consts["CHRs"], consts["RB"]
    NS = consts["NS"]
    RPtot, IDXW = consts["RPtot"], consts["IDXW"]
    CH_OF = [max(h for h in range(NCH) if CB[h] <= t) for t in range(NTILES)]
    off8 = consts["off8"]

    nc = bacc.Bacc("TRN2", target_bir_lowering=False, debug=False, num_devices=C, num_swdge_queues=4)

    xpairT = nc.dram_tensor("xpairT", [IN, RPtot], bf16, kind="ExternalInput").ap()
    xownT = nc.dram_tensor("xownT", [IN, NL], bf16, kind="ExternalInput").ap()
    wkm = nc.dram_tensor("wkm", [IN, NT * ET * 2 * D], bf16, kind="ExternalInput").ap()
    wqa = nc.dram_tensor("wqa", [IN, NT * D], bf16, kind="ExternalInput").ap()
    wa = nc.dram_tensor("wa", [D, NT * D], bf16, kind="ExternalInput").ap()
    degf = nc.dram_tensor("degf", [128, NTILES], f32, kind="ExternalInput").ap()
    ntc = nc.dram_tensor("ntc", [128, NTILES], f32, kind="ExternalInput").ap()
    iotaf = nc.dram_tensor("iotaf", [128, WMAX], f32, kind="ExternalInput").ap()
    kmidx = nc.dram_tensor("kmidx", [16, IDXW], i16, kind="ExternalInput").ap()
    outp = nc.dram_tensor("outp", [NL, D], bf16, kind="ExternalOutput").ap()
    kmtab = [
        nc.dram_tensor(f"kmtab{h}", [CHRs[h], 2 * D], bf16, kind="Internal").ap()
        for h in range(NCH)
    ]

    with tile.TileContext(nc) as tc:
        with tc.tile_pool(name="const", bufs=1) as constp, \
             tc.tile_pool(name="stage", bufs=6) as stage, \
             tc.tile_pool(name="work", bufs=3) as work, \
             tc.tile_pool(name="gtpool", bufs=5) as gtpool, \
             tc.tile_pool(name="npsum", bufs=3, space="PSUM") as npsum, \
             tc.tile_pool(name="tpsum", bufs=3, space="PSUM") as tpsum, \
             tc.tile_pool(name="mpsum", bufs=2, space="PSUM") as mpsum:

            # ---- persistent constants ----
            kmidx_s = constp.tile([128, IDXW], i16, name="kmidx_s", tag="kmidx_s")
            for k in range(8):
                nc.sync.dma_start(out=kmidx_s[16 * k : 16 * (k + 1), :], in_=kmidx[:, :])
            wkm_s = constp.tile([IN, NT * ET * 2 * D], bf16, name="wkm_s", tag="wkm_s")
            nc.sync.dma_start(out=wkm_s[:], in_=wkm[:, :])
            wqa_s = constp.tile([IN, NT * D], bf16, name="wqa_s", tag="wqa_s")
            nc.sync.dma_start(out=wqa_s[:], in_=wqa[:, :])
            wa_s = constp.tile([D, NT * D], bf16, name="wa_s", tag="wa_s")
            nc.sync.dma_start(out=wa_s[:], in_=wa[:, :])
            xown_s = constp.tile([IN, NL], bf16, name="xown_s", tag="xown_s")
            nc.sync.dma_start(out=xown_s[:], in_=xownT[:, :])
            oneh_s = constp.tile([128, NTILES * NT], f32, name="oneh_s", tag="oneh_s")
            degf_s = constp.tile([128, NTILES], f32, name="degf_s", tag="degf_s")
            nc.sync.dma_start(out=degf_s[:], in_=degf[:, :])
            ntc_s = constp.tile([128, NTILES], f32, name="ntc_s", tag="ntc_s")
            nc.sync.dma_start(out=ntc_s[:], in_=ntc[:, :])
            iotaf_s = constp.tile([128, WMAX], f32, name="iotaf_s", tag="iotaf_s")
            nc.sync.dma_start(out=iotaf_s[:], in_=iotaf[:, :])
            for t4 in range(NT):
                nc.vector.tensor_scalar(
                    out=oneh_s[:].rearrange("p (t f) -> p t f", t=NTILES)[:, :, t4],
                    in0=ntc_s[:],
                    scalar1=float(t4),
                    scalar2=None,
                    op0=mybir.AluOpType.is_equal,
                )
            abias_f = constp.tile(
                [128, NTILES * WMAX], f32, name="abias_f", tag="abias_f"
            )
            ab3 = abias_f[:].rearrange("p (t w) -> p t w", t=NTILES)
            nc.vector.tensor_tensor(
                out=ab3,
                in0=iotaf_s[:].unsqueeze(1).to_broadcast([128, NTILES, WMAX]),
                in1=degf_s[:].unsqueeze(2).to_broadcast([128, NTILES, WMAX]),
                op=mybir.AluOpType.is_lt,
            )
            nc.vector.tensor_scalar(
                out=abias_f[:],
                in0=abias_f[:],
                scalar1=1.0,
                scalar2=-NEG,
                op0=mybir.AluOpType.subtract,
                op1=mybir.AluOpType.mult,
            )
            ident = constp.tile([128, 128], bf16, name="ident", tag="ident")
            make_identity(nc, ident[:])
            qall = constp.tile([128, NTILES * D], bf16, name="qall", tag="qall")

            # ---- Q phase: typed projection of own-node x (host-pregathered) ----
            def q_phase():
                for t in range(NTILES):
                    qp = mpsum.tile([128, NT * D], f32, space="PSUM", name=f"qp{t}", tag="mp")
                    nc.tensor.matmul(
                        qp[:], lhsT=xown_s[:, t * 128 : (t + 1) * 128],
                        rhs=wqa_s[:], start=True, stop=True,
                    )
                    qtmp = work.tile([128, NT * D], f32, name=f"qtmp{t}", tag="qtmp")
                    ohb = (
                        oneh_s[:]
                        .rearrange("p (t f) -> p t f", t=NTILES)[:, t]
                        .unsqueeze(1)
                        .to_broadcast([128, D, NT])
                    )
                    nc.vector.tensor_tensor(
                        out=qtmp[:].rearrange("p (t d) -> p d t", t=NT),
                        in0=qp[:].rearrange("p (t d) -> p d t", t=NT),
                        in1=ohb,
                        op=mybir.AluOpType.mult,
                    )
                    with nc.allow_low_precision(reason="4-way one-hot select to bf16"):
                        nc.vector.tensor_reduce(
                            out=qall[:, t * D : (t + 1) * D],
                            in_=qtmp[:].rearrange("p (t d) -> p d t", t=NT),
                            axis=mybir.AxisListType.X,
                            op=mybir.AluOpType.add,
                        )

            # ---- pair-table build per chunk (dense: host-pregathered xpairT) ----
            # Groups are packed back-to-back (no 128-row alignment); each
            # 128-row output tile may span several (ntype, etype) groups, so
            # it gets one partial-M matmul per overlapped group.
            def node_chunk_emitters(h):
                bounds = []  # (start_row, end_row, g) for nonempty groups
                for g in range(NT * ET):
                    if int(R[h, g]) > 0:
                        bounds.append((int(gbase[h, g]), int(gbase[h, g]) + int(R[h, g]), g))
                GT = bounds[-1][1]  # real rows (64-aligned)
                n_tiles = (GT + 127) // 128
                SLAB = 16
                emitters = []
                for s0 in range(0, n_tiles, SLAB):
                    emitters.append(
                        lambda s0=s0: node_slab(h, bounds, GT, n_tiles, SLAB, s0)
                    )
                return emitters

            def node_slab(h, bounds, GT, n_tiles, SLAB, s0):
                nb = min(SLAB, n_tiles - s0)
                row0 = s0 * 128
                rows = min(GT, (s0 + nb) * 128) - row0
                lhs_s = stage.tile([IN, SLAB * 128], bf16, name=f"lhs_{h}_{s0}", tag="lhs")
                nc.sync.dma_start(
                    out=lhs_s[:, :rows],
                    in_=xpairT[:, int(RB[h]) + row0 : int(RB[h]) + row0 + rows],
                )
                slab = stage.tile(
                    [128, SLAB, 2 * D], bf16, name=f"slab_{h}_{s0}", tag="slab"
                )
                for i in range(0, nb, 4):
                    nn = min(4, nb - i)
                    km_p = npsum.tile(
                        [128, 512], f32, space="PSUM", name=f"km_p{h}_{s0}_{i}", tag="km_p"
                    )
                    covers = []
                    for j in range(nn):
                        t0 = row0 + (i + j) * 128  # tile's first table row
                        covers.append(min(128, GT - t0))
                        for gs, ge, g in bounds:
                            lo, hi = max(gs, t0), min(ge, t0 + 128)
                            if lo >= hi:
                                continue
                            nc.tensor.matmul(
                                km_p[lo - t0 : hi - t0, j * 128 : (j + 1) * 128],
                                lhsT=lhs_s[:, (i + j) * 128 + lo - t0 : (i + j) * 128 + hi - t0],
                                rhs=wkm_s[:, g * 128 : (g + 1) * 128],
                                start=True,
                                stop=True,
                            )
                    if covers[-1] == 128:
                        nc.any.tensor_copy(
                            out=slab[:, i : i + nn],
                            in_=km_p[:].rearrange("p (a d) -> p a d", a=4)[:, :nn],
                        )
                    else:
                        for j in range(nn):
                            nc.any.tensor_copy(
                                out=slab[: covers[j], i + j],
                                in_=km_p[: covers[j], j * 128 : (j + 1) * 128],
                            )
                # p-major DRAM write: row' = p*NS_h + a, contiguous per partition
                # (on the Scalar HWDGE ring, separate from the Sync lhs loads)
                km3 = kmtab[h][:, :].rearrange("(p a) d -> p a d", p=128)
                full = rows // 128
                if full:
                    nc.scalar.dma_start(
                        out=km3[:, s0 : s0 + full, :], in_=slab[:, :full]
                    )
                if rows % 128:
                    pr = rows % 128
                    nc.scalar.dma_start(
                        out=km3[:pr, s0 + full, :], in_=slab[:pr, full]
                    )

            # ---- phase 3: per node-tile softmax + aggregation ----
            def p3_tile(t):
                h = CH_OF[t]
                wt = int(W[t])
                n_idx = 128 * wt
                o8 = int(off8[t])
                gt = gtpool.tile([128, WMAX, 2 * D], bf16, name=f"gt{t}", tag="gt")
                # split each tile's gather across both SWDGE queues: the two
                # Q7 core pairs generate descriptors concurrently (~2x)
                wh = wt // 2
                if wh:
                    nc.gpsimd.dma_gather(
                        out_ap=gt[:, :wh],
                        in_ap=kmtab[h][:, :],
                        idxs_ap=kmidx_s[:, o8 : o8 + 8 * wh],
                        num_idxs=128 * wh,
                        num_idxs_reg=128 * wh,
                        elem_size=2 * D,
                        single_packet=False,
                        queue_num=0,
                    )
                    nc.gpsimd.dma_gather(
                        out_ap=gt[:, wh:wt],
                        in_ap=kmtab[h][:, :],
                        idxs_ap=kmidx_s[:, o8 + 8 * wh : o8 + 8 * wt],
                        num_idxs=128 * (wt - wh),
                        num_idxs_reg=128 * (wt - wh),
                        elem_size=2 * D,
                        single_packet=False,
                        queue_num=1,
                    )
                else:
                    nc.gpsimd.dma_gather(
                        out_ap=gt[:, :wt],
                        in_ap=kmtab[h][:, :],
                        idxs_ap=kmidx_s[:, o8 : o8 + 8 * wt],
                        num_idxs=n_idx,
                        num_idxs_reg=n_idx,
                        elem_size=2 * D,
                        single_packet=False,
                        queue_num=t % 2,
                    )
                aprod = work.tile([128, WMAX, D], bf16, name=f"aprod{t}", tag="aprod")
                qb = qall[:, t * D : (t + 1) * D].unsqueeze(1).to_broadcast([128, wt, D])
                with nc.allow_low_precision(reason="bf16 attention products"):
                    nc.vector.tensor_tensor(
                        out=aprod[:, :wt], in0=gt[:, :wt, :D], in1=qb, op=mybir.AluOpType.mult
                    )
                am = work.tile([128, H, WMAX], f32, name=f"am{t}", tag="am")
                nc.vector.tensor_reduce(
                    out=am[:, :, :wt],
                    in_=aprod[:, :wt].rearrange("p w (h d) -> p h w d", h=H),
                    axis=mybir.AxisListType.X,
                    op=mybir.AluOpType.add,
                )
                amb = work.tile([128, H, WMAX], f32, name=f"amb{t}", tag="amb")
                bb = (
                    abias_f[:]
                    .rearrange("p (t w) -> p t w", t=NTILES)[:, t, :wt]
                    .unsqueeze(1)
                    .to_broadcast([128, H, wt])
                )
                nc.vector.tensor_tensor(
                    out=amb[:, :, :wt], in0=am[:, :, :wt], in1=bb, op=mybir.AluOpType.add
                )
                # softmax without max-subtraction: |a| is bounded well below
                # f32 exp overflow, and pads carry a -1e30 bias -> exp == 0.
                ex = work.tile([128, H, WMAX], bf16, name=f"ex{t}", tag="ex")
                nc.scalar.activation(
                    out=ex[:, :, :wt], in_=amb[:, :, :wt],
                    func=mybir.ActivationFunctionType.Exp,
                )
                den = work.tile([128, H], f32, name=f"den{t}", tag="den")
                nc.vector.tensor_reduce(
                    out=den[:], in_=ex[:, :, :wt],
                    axis=mybir.AxisListType.X, op=mybir.AluOpType.add,
                )
                rden = work.tile([128, H], f32, name=f"rden{t}", tag="rden")
                nc.vector.reciprocal(out=rden[:], in_=den[:])
                mprod = work.tile([128, H, HS, WMAX], bf16, name=f"mprod{t}", tag="mprod")
                mpart = gt[:, :wt, D:].rearrange("p w (h d) -> p h d w", h=H)
                ab2 = ex[:, :, :wt].unsqueeze(2).to_broadcast([128, H, HS, wt])
                nc.vector.tensor_tensor(
                    out=mprod[:, :, :, :wt], in0=mpart, in1=ab2, op=mybir.AluOpType.mult
                )
                hm = work.tile([128, D], f32, name=f"hm{t}", tag="hm")
                nc.vector.tensor_reduce(
                    out=hm[:].rearrange("p (h d) -> p h d", h=H),
                    in_=mprod[:, :, :, :wt],
                    axis=mybir.AxisListType.X,
                    op=mybir.AluOpType.add,
                )
                hm2 = work.tile([128, D], bf16, name=f"hm2{t}", tag="hm2")
                nc.vector.tensor_tensor(
                    out=hm2[:].rearrange("p (h d) -> p h d", h=H),
                    in0=hm[:].rearrange("p (h d) -> p h d", h=H),
                    in1=rden[:].unsqueeze(2).to_broadcast([128, H, HS]),
                    op=mybir.AluOpType.mult,
                )
                tp = tpsum.tile([128, 128], bf16, space="PSUM", name=f"tph{t}", tag="tp")
                nc.tensor.transpose(out=tp[:D, :], in_=hm2[:], identity=ident[:])
                hT = work.tile([D, 128], bf16, name=f"hT{t}", tag="hT")
                nc.any.tensor_copy(out=hT[:], in_=tp[:D, :])
                o4 = mpsum.tile([128, NT * D], f32, space="PSUM", name=f"o4_{t}", tag="mp")
                nc.tensor.matmul(o4[:], lhsT=hT[:], rhs=wa_s[:], start=True, stop=True)
                osel = work.tile([128, NT * D], f32, name=f"osel{t}", tag="osel")
                ohb = (
                    oneh_s[:]
                    .rearrange("p (t f) -> p t f", t=NTILES)[:, t]
                    .unsqueeze(1)
                    .to_broadcast([128, D, NT])
                )
                nc.vector.tensor_tensor(
                    out=osel[:].rearrange("p (t d) -> p d t", t=NT),
                    in0=o4[:].rearrange("p (t d) -> p d t", t=NT),
                    in1=ohb,
                    op=mybir.AluOpType.mult,
                )
                ot = work.tile([128, D], bf16, name=f"ot{t}", tag="ot")
                with nc.allow_low_precision(reason="4-way one-hot select to bf16 output"):
                    nc.vector.tensor_reduce(
                        out=ot[:],
                        in_=osel[:].rearrange("p (t d) -> p d t", t=NT),
                        axis=mybir.AxisListType.X,
                        op=mybir.AluOpType.add,
                    )
                nc.sync.dma_start(out=outp[t * 128 : (t + 1) * 128, :], in_=ot[:])

            # emission order = scheduler priority: chunk 0's table build first
            # (the first gather depends on it), then the Q phase, then
            # interleave later chunks' build slabs with phase 3 of the
            # already-built chunks.  The Pool engine only runs the p3 gathers;
            # everything else hides under its serial descriptor generation.
            for em in node_chunk_emitters(0):
                em()
            for em in node_chunk_emitters(1):
                em()
            q_phase()
            for t in range(CB[0], CB[1]):
                p3_tile(t)
            for h in range(2, NCH):
                slabs = node_chunk_emitters(h)
                tiles = list(range(CB[h - 1], CB[h]))
                ns, ntl = len(slabs), len(tiles)
                si = ti = 0
                while si < ns or ti < ntl:
                    take = (si + 1) * ntl <= (ti + 1) * ns
                    if si < ns and (take or ti >= ntl):
                        slabs[si]()
                        si += 1
                    else:
                        p3_tile(tiles[ti])
                        ti += 1
            for t in range(CB[NCH - 1], NTILES):
                p3_tile(t)

    nc.compile()
    return nc


LAST_RESULTS = None


def kernel(x, ntype, etype, src, dst, Wk, Wq, Wv, Wa, rel_att, rel_msg, rel_pri):
    import os

    from concourse import bass_utils

    cores, consts = _host_prep(x, ntype, etype, src, dst)
    wkm, wqa, wa_all = _fold_weights(Wk, Wq, Wv, Wa, rel_att, rel_msg, rel_pri)

    struct_sig = (
        tuple(consts["W"].tolist()),
        consts["NCH"],
        tuple(consts["CHRs"]),
        tuple(consts["R"].ravel().tolist()),
    )
    if "prog" not in _cache or _cache["prog"][0] != struct_sig:
        _cache["prog"] = (struct_sig, _build_program(consts))
    nc = _cache["prog"][1]

    in_maps = [
        dict(
            xpairT=d["xpairT"], xownT=d["xownT"],
            wkm=wkm, wqa=wqa, wa=wa_all,
            degf=d["degf"], ntc=d["ntc"], iotaf=d["iotaf"], kmidx=d["kmidx"],
        )
        for c, d in enumerate(cores)
    ]
    trace = bool(int(os.environ.get("GNN_TRACE", "0")))
    res = bass_utils.run_bass_kernel_spmd(
        nc,
        in_maps,
        core_ids=list(range(C)),
        trace=trace,
        tmpdir=os.environ.get("GNN_TRACE_DIR") or None,
    )
    global LAST_RESULTS
    LAST_RESULTS = res

    out = np.zeros((N, D), dtype=np.float32)
    own = consts["own_nodes"]
    for c in range(C):
        oc = np.asarray(res.results[c]["outp"], dtype=np.float32)
        m = own[c] >= 0
        out[own[c][m]] = oc[m]
    out[consts["deg"] == 0] = 0.0
    return out


# revision 16
# speedup vs baseline: 1.2876x; 1.2876x over previous
"""HGT-style heterogeneous graph message passing on 8 Trainium2 cores.

v3: Pool-descriptor-minimized variant.  v2's wall was the GpSimd (Pool)
engine serially generating SWDGE descriptors at ~8ns/row for ~162k
gathered rows per core (pair-table x gather + own-node x gather +
per-edge-slot table gather).  v3 ships the x rows that the table build
and Q phase need ALREADY GATHERED AND TRANSPOSED from the host (the
sharding hint's "each device holds its edge slice + gathered src/dst
features"), so the only SWDGE gather left is the irreducible per-edge-slot
fetch of the on-device-computed [k'|m] pair rows (~83k rows/core).

 - Host folds the per-(head, etype) relation transforms into per-(ntype,
   etype) 64x128 weight matrices:  a_e = <k'_src, q_dst>  with
   k' = x @ Wk[nt] @ blockdiag_h(A A^T * pri / sqrt(d)),  m = x @ Wv[nt] @
   blockdiag_h(M).
 - dst nodes are sharded across the 8 cores round-robin by degree rank, so
   all segment ops (softmax max/sum, weighted aggregation) become dense row
   reductions over degree-sorted [128, W_t] tiles.  No collectives.
 - Each core builds its deduplicated (src, etype) pair table [rows, 128] =
   [k' | m] in bf16 with dense matmuls over host-pregathered xpairT slabs
   (no on-device gather, no PE transposes), writes it to DRAM partition-major
   (contiguous HWDGE descriptors), then dma_gather's its edge slots
   (256B/row) — the one remaining Pool-engine cost.
"""

import sys

sys.path.insert(0, "/opt/trn_rl_repo")

import numpy as np
import ml_dtypes

BF16 = ml_dtypes.bfloat16

N, E = 40000, 640000
IN, H, HS = 64, 4, 16
NT, ET = 4, 8
D = H * HS  # 64
C = 8  # cores
NL = 5120  # padded local nodes per core
NTILES = NL // 128  # 40
NEG = -1.0e30

_cache = {}


def _host_prep(x, ntype, etype, src, dst):
    """Returns per-core input arrays + structural constants."""
    x = np.ascontiguousarray(np.asarray(x, dtype=np.float32))
    nt_ = np.asarray(ntype).astype(np.int64)
    et_ = np.asarray(etype).astype(np.int64)
    src = np.asarray(src).astype(np.int64)
    dst = np.asarray(dst).astype(np.int64)

    deg = np.bincount(dst, minlength=N)
    order = np.argsort(-deg, kind="stable")
    ranks = np.empty(N, dtype=np.int64)
    ranks[order] = np.arange(N)
    core_of_node = ranks % C
    local_of_node = ranks // C

    # tile widths (shared across cores): tile t covers global ranks [1024t, 1024(t+1))
    W = np.zeros(NTILES, dtype=np.int64)
    deg_by_rank = deg[order]
    for t in range(NTILES):
        lo, hi = t * 1024, min((t + 1) * 1024, N)
        W[t] = max(int(deg_by_rank[lo:hi].max()) if hi > lo else 1, 1)

    percore = []
    for c in range(C):
        ei = np.nonzero(core_of_node[dst] == c)[0]
        ld = local_of_node[dst[ei]]
        o = np.argsort(ld, kind="stable")
        percore.append((ei[o], ld[o]))

    # table chunks: small leading chunks so the first gathers start early;
    # each chunk's padded pair count must stay < 32000 (int16 gather idxs)
    CB = [0, 1, 2, 4, 7, 11, 16, 22, 28, 34, NTILES]
    NCH = len(CB) - 1
    chunk_of_tile = np.zeros(NTILES, dtype=np.int64)
    for h in range(NCH):
        chunk_of_tile[CB[h] : CB[h + 1]] = h
    cnts = np.zeros((C, NCH, NT * ET), dtype=np.int64)
    pair_data = []
    for c in range(C):
        ei, ld = percore[c]
        ch_of = chunk_of_tile[ld // 128]
        key = src[ei] * ET + et_[ei]
        chunk_pairs = []
        for h in range(NCH):
            uk = np.unique(key[ch_of == h])  # sorted keys
            g = nt_[uk // ET] * ET + (uk % ET)
            np.add.at(cnts[c, h], g, 1)
            chunk_pairs.append((uk, g))
        pair_data.append(chunk_pairs)
    # 128-row group alignment: every 128-row build tile is exactly one
    # (ntype, etype) group -> single matmul per tile, batched copies
    R = 128 * ((cnts.max(axis=0) + 127) // 128)  # [NCH, 32]
    CHRs = R.sum(axis=1)
    assert CHRs.max() < 32000, CHRs

    gbase = np.zeros((NCH, NT * ET), dtype=np.int64)
    for h in range(NCH):
        gbase[h] = np.concatenate(([0], np.cumsum(R[h])[:-1]))
    CHRs = [int(v) for v in CHRs]
    RB = np.concatenate(([0], np.cumsum(CHRs)[:-1])).astype(np.int64)
    RPtot = int(sum(CHRs))
    NS = [v // 128 for v in CHRs]  # p-major columns per chunk

    IDX8 = (8 * W).astype(np.int64)
    off8 = np.concatenate(([0], np.cumsum(IDX8)[:-1]))
    offw = np.concatenate(([0], np.cumsum(W)[:-1]))
    IDXW = int(IDX8.sum())
    ABW = int(W.sum())

    cores = []
    own_nodes = np.full((C, NL), -1, dtype=np.int64)
    for c in range(C):
        ei, ld = percore[c]
        etile = ld // 128
        key = src[ei] * ET + et_[ei]

        ownc = order[c::C]
        own_nodes[c, : len(ownc)] = ownc

        ch_of = chunk_of_tile[etile]

        rowid_of_edge = np.zeros(len(ei), dtype=np.int64)
        xp_node = np.full(RPtot, -1, dtype=np.int64)
        for h in range(NCH):
            uk, g = pair_data[c][h]  # uk sorted by key; g aligned
            po = np.argsort(g, kind="stable")
            gs = g[po]
            base_in_g = np.concatenate(
                ([0], np.cumsum(np.bincount(gs, minlength=NT * ET))[:-1])
            )
            rows_po = gbase[h][gs] + (np.arange(len(uk)) - base_in_g[gs])
            row_of_uk = np.empty(len(uk), dtype=np.int64)
            row_of_uk[po] = rows_po
            xp_node[RB[h] + row_of_uk] = uk // ET
            sel = np.nonzero(ch_of == h)[0]
            r = row_of_uk[np.searchsorted(uk, key[sel])]
            # p-major remap: DRAM row' = (r % 128) * NS_h + r // 128
            rowid_of_edge[sel] = (r % 128) * NS[h] + r // 128

        # host-pregathered, transposed x rows for the pair table build
        pn = np.where(xp_node >= 0, xp_node, 0)
        xpairT = np.ascontiguousarray(x[pn].astype(BF16).T)  # [64, RPtot]
        on = np.where(own_nodes[c] >= 0, own_nodes[c], 0)
        xownT = np.ascontiguousarray(x[on].astype(BF16).T)  # [64, NL]

        cnt = np.bincount(ld, minlength=NL)
        starts = np.concatenate(([0], np.cumsum(cnt)[:-1]))
        jpos = np.arange(len(ei)) - starts[ld]
        p_of = ld % 128

        kmidx = np.zeros((16, IDXW), dtype=np.int16)
        for t in range(NTILES):
            wt = int(W[t])
            sel = np.nonzero(etile == t)[0]
            M = np.zeros((128, wt), dtype=np.int16)
            M[p_of[sel], jpos[sel]] = rowid_of_edge[sel].astype(np.int16)
            idsl = M.T.ravel()  # list position k = j*128 + p
            kmidx[:, int(off8[t]) : int(off8[t]) + 8 * wt] = idsl.reshape(
                8 * wt, 16
            ).T

        # abias/oneh are built on device from per-node degree and type
        degf = np.ascontiguousarray(
            cnt.astype(np.float32).reshape(NTILES, 128).T
        )  # [128, NTILES]
        ntv = np.where(own_nodes[c] >= 0, nt_[on], 255).astype(np.float32)
        ntc = np.ascontiguousarray(ntv.reshape(NTILES, 128).T)  # [128, NTILES]

        cores.append(
            dict(
                xpairT=xpairT, xownT=xownT,
                degf=degf, ntc=ntc, kmidx=kmidx,
                iotaf=np.tile(np.arange(int(W.max()), dtype=np.float32), (128, 1)),
            )
        )

    consts = dict(
        W=W, WMAX=int(W.max()), NCH=NCH, CB=CB, R=R, gbase=gbase, CHRs=CHRs,
        NS=NS, RB=RB, RPtot=RPtot, IDXW=IDXW, ABW=ABW, off8=off8, offw=offw,
        own_nodes=own_nodes, deg=deg,
    )
    return cores, consts


def _fold_weights(Wk, Wq, Wv, Wa, rel_att, rel_msg, rel_pri):
    Wk = np.asarray(Wk, np.float64)
    Wq = np.asarray(Wq, np.float64)
    Wv = np.asarray(Wv, np.float64)
    Wa = np.asarray(Wa, np.float64)
    rel_att = np.asarray(rel_att, np.float64)
    rel_msg = np.asarray(rel_msg, np.float64)
    rel_pri = np.asarray(rel_pri, np.float64)
    sd = float(np.sqrt(np.float32(HS)))

    wkm = np.zeros((IN, NT * ET, 2, D), np.float64)
    for nt in range(NT):
        for et in range(ET):
            Batt = np.zeros((D, D))
            Bmsg = np.zeros((D, D))
            for h in range(H):
                A = rel_att[h, et]
                Batt[h * HS : (h + 1) * HS, h * HS : (h + 1) * HS] = (
                    A @ A.T * rel_pri[h, et] / sd
                )
                Bmsg[h * HS : (h + 1) * HS, h * HS : (h + 1) * HS] = rel_msg[h, et]
            g = nt * ET + et
            wkm[:, g, 0] = Wk[nt] @ Batt
            wkm[:, g, 1] = Wv[nt] @ Bmsg
    wkm = wkm.reshape(IN, NT * ET * 2 * D).astype(BF16)
    wkm = np.ascontiguousarray(wkm)  # [64, 4096]
    wqa = np.concatenate([Wq[t] for t in range(NT)], axis=1).astype(BF16)  # [64, 256]
    wa_all = np.concatenate([Wa[t] for t in range(NT)], axis=1).astype(BF16)
    return wkm, wqa, wa_all


def _build_program(consts):
    import concourse.mybir as mybir
    import concourse.tile as tile
    from concourse import bacc
    from concourse.masks import make_identity

    f32 = mybir.dt.float32
    bf16 = mybir.dt.bfloat16
    i16 = mybir.dt.int16
    W = consts["W"]
    WMAX = consts["WMAX"]
    NCH, CB = consts["NCH"], consts["CB"]
    R, gbase, CHRs, RB = consts["R"], consts["gbase"], consts["CHRs"], consts["RB"]
    NS = consts["NS"]
    RPtot, IDXW = consts["RPtot"], consts["IDXW"]
    CH_OF = [max(h for h in range(NCH) if CB[h] <= t) for t in range(NTILES)]
    off8 = consts["off8"]

    nc = bacc.Bacc("TRN2", target_bir_lowering=False, debug=False, num_devices=C, num_swdge_queues=4)

    xpairT = nc.dram_tensor("xpairT", [IN, RPtot], bf16, kind="ExternalInput").ap()
    xownT = nc.dram_tensor("xownT", [IN, NL], bf16, kind="ExternalInput").ap()
    wkm = nc.dram_tensor("wkm", [IN, NT * ET * 2 * D], bf16, kind="ExternalInput").ap()
    wqa = nc.dram_tensor("wqa", [IN, NT * D], bf16, kind="ExternalInput").ap()
    wa = nc.dram_tensor("wa", [D, NT * D], bf16, kind="ExternalInput").ap()
    degf = nc.dram_tensor("degf", [128, NTILES], f32, kind="ExternalInput").ap()
    ntc = nc.dram_tensor("ntc", [128, NTILES], f32, kind="ExternalInput").ap()
    iotaf = nc.dram_tensor("iotaf", [128, WMAX], f32, kind="ExternalInput").ap()
    kmidx = nc.dram_tensor("kmidx", [16, IDXW], i16, kind="ExternalInput").ap()
    outp = nc.dram_tensor("outp", [NL, D], bf16, kind="ExternalOutput").ap()
    kmtab = [
        nc.dram_tensor(f"kmtab{h}", [CHRs[h], 2 * D], bf16, kind="Internal").ap()
        for h in range(NCH)
    ]

    with tile.TileContext(nc) as tc:
        with tc.tile_pool(name="const", bufs=1) as constp, \
             tc.tile_pool(name="stage", bufs=4) as stage, \
             tc.tile_pool(name="work", bufs=3) as work, \
             tc.tile_pool(name="gtpool", bufs=5) as gtpool, \
             tc.tile_pool(name="npsum", bufs=3, space="PSUM") as npsum, \
             tc.tile_pool(name="tpsum", bufs=3, space="PSUM") as tpsum, \
             tc.tile_pool(name="mpsum", bufs=2, space="PSUM") as mpsum:

            # ---- persistent constants ----
            kmidx_s = constp.tile([128, IDXW], i16, name="kmidx_s", tag="kmidx_s")
            for k in range(8):
                nc.sync.dma_start(out=kmidx_s[16 * k : 16 * (k + 1), :], in_=kmidx[:, :])
            wkm_s = constp.tile([IN, NT * ET * 2 * D], bf16, name="wkm_s", tag="wkm_s")
            nc.sync.dma_start(out=wkm_s[:], in_=wkm[:, :])
            wqa_s = constp.tile([IN, NT * D], bf16, name="wqa_s", tag="wqa_s")
            nc.sync.dma_start(out=wqa_s[:], in_=wqa[:, :])
            wa_s = constp.tile([D, NT * D], bf16, name="wa_s", tag="wa_s")
            nc.sync.dma_start(out=wa_s[:], in_=wa[:, :])
            xown_s = constp.tile([IN, NL], bf16, name="xown_s", tag="xown_s")
            nc.sync.dma_start(out=xown_s[:], in_=xownT[:, :])
            oneh_s = constp.tile([128, NTILES * NT], f32, name="oneh_s", tag="oneh_s")
            degf_s = constp.tile([128, NTILES], f32, name="degf_s", tag="degf_s")
            nc.sync.dma_start(out=degf_s[:], in_=degf[:, :])
            ntc_s = constp.tile([128, NTILES], f32, name="ntc_s", tag="ntc_s")
            nc.sync.dma_start(out=ntc_s[:], in_=ntc[:, :])
            iotaf_s = constp.tile([128, WMAX], f32, name="iotaf_s", tag="iotaf_s")
            nc.sync.dma_start(out=iotaf_s[:], in_=iotaf[:, :])
            for t4 in range(NT):
                nc.vector.tensor_scalar(
                    out=oneh_s[:].rearrange("p (t f) -> p t f", t=NTILES)[:, :, t4],
                    in0=ntc_s[:],
                    scalar1=float(t4),
                    scalar2=None,
                    op0=mybir.AluOpType.is_equal,
                )
            abias_f = constp.tile(
                [128, NTILES * WMAX], f32, name="abias_f", tag="abias_f"
            )
            ab3 = abias_f[:].rearrange("p (t w) -> p t w", t=NTILES)
            nc.vector.tensor_tensor(
                out=ab3,
                in0=iotaf_s[:].unsqueeze(1).to_broadcast([128, NTILES, WMAX]),
                in1=degf_s[:].unsqueeze(2).to_broadcast([128, NTILES, WMAX]),
                op=mybir.AluOpType.is_lt,
            )
            nc.vector.tensor_scalar(
                out=abias_f[:],
                in0=abias_f[:],
                scalar1=1.0,
                scalar2=-NEG,
                op0=mybir.AluOpType.subtract,
                op1=mybir.AluOpType.mult,
            )
            ident = constp.tile([128, 128], bf16, name="ident", tag="ident")
            make_identity(nc, ident[:])
            qall = constp.tile([128, NTILES * D], bf16, name="qall", tag="qall")

            # ---- Q phase: typed projection of own-node x (host-pregathered) ----
            def q_phase():
                for t in range(NTILES):
                    qp = mpsum.tile([128, NT * D], f32, space="PSUM", name=f"qp{t}", tag="mp")
                    nc.tensor.matmul(
                        qp[:], lhsT=xown_s[:, t * 128 : (t + 1) * 128],
                        rhs=wqa_s[:], start=True, stop=True,
                    )
                    qtmp = work.tile([128, NT * D], f32, name=f"qtmp{t}", tag="qtmp")
                    ohb = (
                        oneh_s[:]
                        .rearrange("p (t f) -> p t f", t=NTILES)[:, t]
                        .unsqueeze(1)
                        .to_broadcast([128, D, NT])
                    )
                    nc.vector.tensor_tensor(
                        out=qtmp[:].rearrange("p (t d) -> p d t", t=NT),
                        in0=qp[:].rearrange("p (t d) -> p d t", t=NT),
                        in1=ohb,
                        op=mybir.AluOpType.mult,
                    )
                    with nc.allow_low_precision(reason="4-way one-hot select to bf16"):
                        nc.vector.tensor_reduce(
                            out=qall[:, t * D : (t + 1) * D],
                            in_=qtmp[:].rearrange("p (t d) -> p d t", t=NT),
                            axis=mybir.AxisListType.X,
                            op=mybir.AluOpType.add,
                        )

            # ---- pair-table build per chunk (dense: host-pregathered xpairT) ----
            # Groups are packed back-to-back (no 128-row alignment); each
            # 128-row output tile may span several (ntype, etype) groups, so
            # it gets one partial-M matmul per overlapped group.
            def node_chunk_emitters(h):
                # 128-aligned groups: build tile i is exactly group g_of[i]
                g_of = []
                for g in range(NT * ET):
                    g_of += [g] * (int(R[h, g]) // 128)
                n_tiles = len(g_of)
                assert n_tiles * 128 == int(CHRs[h])
                SLAB = 16
                emitters = []
                for s0 in range(0, n_tiles, SLAB):
                    emitters.append(
                        lambda s0=s0: node_slab(h, g_of, n_tiles, SLAB, s0)
                    )
                return emitters

            def node_slab(h, g_of, n_tiles, SLAB, s0):
                nb = min(SLAB, n_tiles - s0)
                row0 = s0 * 128
                rows = nb * 128
                lhs_s = stage.tile([IN, SLAB * 128], bf16, name=f"lhs_{h}_{s0}", tag="lhs")
                nc.sync.dma_start(
                    out=lhs_s[:, :rows],
                    in_=xpairT[:, int(RB[h]) + row0 : int(RB[h]) + row0 + rows],
                )
                slab = stage.tile(
                    [128, SLAB, 2 * D], bf16, name=f"slab_{h}_{s0}", tag="slab"
                )
                for i in range(0, nb, 4):
                    nn = min(4, nb - i)
                    km_p = npsum.tile(
                        [128, 512], f32, space="PSUM", name=f"km_p{h}_{s0}_{i}", tag="km_p"
                    )
                    for j in range(nn):
                        g = g_of[s0 + i + j]
                        nc.tensor.matmul(
                            km_p[:, j * 128 : (j + 1) * 128],
                            lhsT=lhs_s[:, (i + j) * 128 : (i + j + 1) * 128],
                            rhs=wkm_s[:, g * 128 : (g + 1) * 128],
                            start=True,
                            stop=True,
                        )
                    nc.any.tensor_copy(
                        out=slab[:, i : i + nn],
                        in_=km_p[:].rearrange("p (a d) -> p a d", a=4)[:, :nn],
                    )
                # p-major DRAM write: row' = p*NS_h + a, contiguous per partition
                km3 = kmtab[h][:, :].rearrange("(p a) d -> p a d", p=128)
                nc.sync.dma_start(out=km3[:, s0 : s0 + nb, :], in_=slab[:, :nb])

            # ---- phase 3: per node-tile softmax + aggregation ----
            def p3_tile(t):
                h = CH_OF[t]
                wt = int(W[t])
                n_idx = 128 * wt
                o8 = int(off8[t])
                gt = gtpool.tile([128, WMAX, 2 * D], bf16, name=f"gt{t}", tag="gt")
                # split each tile's gather across both SWDGE queues: the two
                # Q7 core pairs generate descriptors concurrently (~2x)
                wh = wt // 2
                if wh:
                    nc.gpsimd.dma_gather(
                        out_ap=gt[:, :wh],
                        in_ap=kmtab[h][:, :],
                        idxs_ap=kmidx_s[:, o8 : o8 + 8 * wh],
                        num_idxs=128 * wh,
                        num_idxs_reg=128 * wh,
                        elem_size=2 * D,
                        single_packet=False,
                        queue_num=0,
                    )
                    nc.gpsimd.dma_gather(
                        out_ap=gt[:, wh:wt],
                        in_ap=kmtab[h][:, :],
                        idxs_ap=kmidx_s[:, o8 + 8 * wh : o8 + 8 * wt],
                        num_idxs=128 * (wt - wh),
                        num_idxs_reg=128 * (wt - wh),
                        elem_size=2 * D,
                        single_packet=False,
                        queue_num=1,
                    )
                else:
                    nc.gpsimd.dma_gather(
                        out_ap=gt[:, :wt],
                        in_ap=kmtab[h][:, :],
                        idxs_ap=kmidx_s[:, o8 : o8 + 8 * wt],
                        num_idxs=n_idx,
                        num_idxs_reg=n_idx,
                        elem_size=2 * D,
                        single_packet=False,
                        queue_num=t % 2,
                    )
                aprod = work.tile([128, WMAX, D], bf16, name=f"aprod{t}", tag="aprod")
                qb = qall[:, t * D : (t + 1) * D].unsqueeze(1).to_broadcast([128, wt, D])
                with nc.allow_low_precision(reason="bf16 attention products"):
                    nc.vector.tensor_tensor(
                        out=aprod[:, :wt], in0=gt[:, :wt, :D], in1=qb, op=mybir.AluOpType.mult
                    )
                am = work.tile([128, H, WMAX], f32, name=f"am{t}", tag="am")
                nc.vector.tensor_reduce(
                    out=am[:, :, :wt],
                    in_=aprod[:, :wt].rearrange("p w (h d) -> p h w d", h=H),
                    axis=mybir.AxisListType.X,
                    op=mybir.AluOpType.add,
                )
                amb = work.tile([128, H, WMAX], f32, name=f"amb{t}", tag="amb")
                bb = (
                    abias_f[:]
                    .rearrange("p (t w) -> p t w", t=NTILES)[:, t, :wt]
                    .unsqueeze(1)
                    .to_broadcast([128, H, wt])
                )
                nc.vector.tensor_tensor(
                    out=amb[:, :, :wt], in0=am[:, :, :wt], in1=bb, op=mybir.AluOpType.add
                )
                # softmax without max-subtraction: |a| is bounded well below
                # f32 exp overflow, and pads carry a -1e30 bias -> exp == 0.
                ex = work.tile([128, H, WMAX], bf16, name=f"ex{t}", tag="ex")
                nc.scalar.activation(
                    out=ex[:, :, :wt], in_=amb[:, :, :wt],
                    func=mybir.ActivationFunctionType.Exp,
                )
                den = work.tile([128, H], f32, name=f"den{t}", tag="den")
                nc.vector.tensor_reduce(
                    out=den[:], in_=ex[:, :, :wt],
                    axis=mybir.AxisListType.X, op=mybir.AluOpType.add,
                )
                rden = work.tile([128, H], f32, name=f"rden{t}", tag="rden")
                nc.vector.reciprocal(out=rden[:], in_=den[:])
                mprod = work.tile([128, H, HS, WMAX], bf16, name=f"mprod{t}", tag="mprod")
                mpart = gt[:, :wt, D:].rearrange("p w (h d) -> p h d w", h=H)
                ab2 = ex[:, :, :wt].unsqueeze(2).to_broadcast([128, H, HS, wt])
                nc.vector.tensor_tensor(
                    out=mprod[:, :, :, :wt], in0=mpart, in1=ab2, op=mybir.AluOpType.mult
                )
                hm = work.tile([128, D], f32, name=f"hm{t}", tag="hm")
                nc.vector.tensor_reduce(
                    out=hm[:].rearrange("p (h d) -> p h d", h=H),
                    in_=mprod[:, :, :, :wt],
                    axis=mybir.AxisListType.X,
                    op=mybir.AluOpType.add,
                )
                hm2 = work.tile([128, D], bf16, name=f"hm2{t}", tag="hm2")
                nc.vector.tensor_tensor(
                    out=hm2[:].rearrange("p (h d) -> p h d", h=H),
                    in0=hm[:].rearrange("p (h d) -> p h d", h=H),
                    in1=rden[:].unsqueeze(2).to_broadcast([128, H, HS]),
                    op=mybir.AluOpType.mult,
                )
                tp = tpsum.tile([128, 128], bf16, space="PSUM", name=f"tph{t}", tag="tp")
                nc.tensor.transpose(out=tp[:D, :], in_=hm2[:], identity=ident[:])
                hT = work.tile([D, 128], bf16, name=f"hT{t}", tag="hT")
                nc.any.tensor_copy(out=hT[:], in_=tp[:D, :])
                o4 = mpsum.tile([128, NT * D], f32, space="PSUM", name=f"o4_{t}", tag="mp")
                nc.tensor.matmul(o4[:], lhsT=hT[:], rhs=wa_s[:], start=True, stop=True)
                osel = work.tile([128, NT * D], f32, name=f"osel{t}", tag="osel")
                ohb = (
                    oneh_s[:]
                    .rearrange("p (t f) -> p t f", t=NTILES)[:, t]
                    .unsqueeze(1)
                    .to_broadcast([128, D, NT])
                )
                nc.vector.tensor_tensor(
                    out=osel[:].rearrange("p (t d) -> p d t", t=NT),
                    in0=o4[:].rearrange("p (t d) -> p d t", t=NT),
                    in1=ohb,
                    op=mybir.AluOpType.mult,
                )
                ot = work.tile([128, D], bf16, name=f"ot{t}", tag="ot")
                with nc.allow_low_precision(reason="4-way one-hot select to bf16 output"):
                    nc.vector.tensor_reduce(
                        out=ot[:],
                        in_=osel[:].rearrange("p (t d) -> p d t", t=NT),
                        axis=mybir.AxisListType.X,
                        op=mybir.AluOpType.add,
                    )
                nc.sync.dma_start(out=outp[t * 128 : (t + 1) * 128, :], in_=ot[:])

            # emission order = scheduler priority: chunk 0's table build first
            # (the first gather depends on it), then the Q phase, then
            # interleave later chunks' build slabs with phase 3 of the
            # already-built chunks.  The Pool engine only runs the p3 gathers;
            # everything else hides under its serial descriptor generation.
            for em in node_chunk_emitters(0):
                em()
            for em in node_chunk_emitters(1):
                em()
            q_phase()
            for t in range(CB[0], CB[1]):
                p3_tile(t)
            for h in range(2, NCH):
                slabs = node_chunk_emitters(h)
                tiles = list(range(CB[h - 1], CB[h]))
                ns, ntl = len(slabs), len(tiles)
                si = ti = 0
                while si < ns or ti < ntl:
                    take = (si + 1) * ntl <= (ti + 1) * ns
                    if si < ns and (take or ti >= ntl):
                        slabs[si]()
                        si += 1
                    else:
                        p3_tile(tiles[ti])
                        ti += 1
            for t in range(CB[NCH - 1], NTILES):
                p3_tile(t)

    nc.compile()
    return nc


LAST_RESULTS = None


def kernel(x, ntype, etype, src, dst, Wk, Wq, Wv, Wa, rel_att, rel_msg, rel_pri):
    import os

    from concourse import bass_utils

    cores, consts = _host_prep(x, ntype, etype, src, dst)
    wkm, wqa, wa_all = _fold_weights(Wk, Wq, Wv, Wa, rel_att, rel_msg, rel_pri)

    struct_sig = (
        tuple(consts["W"].tolist()),
        consts["NCH"],
        tuple(consts["CHRs"]),
        tuple(consts["R"].ravel().tolist()),
    )
    if "prog" not in _cache or _cache["prog"][0] != struct_sig:
        _cache["prog"] = (struct_sig, _build_program(consts))
    nc = _cache["prog"][1]

    in_maps = [
        dict(
            xpairT=d["xpairT"], xownT=d["xownT"],
            wkm=wkm, wqa=wqa, wa=wa_all,
            degf=d["degf"], ntc=d["ntc"], iotaf=d["iotaf"], kmidx=d["kmidx"],
        )
        for c, d in enumerate(cores)
    ]
    trace = bool(int(os.environ.get("GNN_TRACE", "0")))
    res = bass_utils.run_bass_kernel_spmd(
        nc,
        in_maps,
        core_ids=list(range(C)),
        trace=trace,
        tmpdir=os.environ.get("GNN_TRACE_DIR") or None,
    )
    global LAST_RESULTS
    LAST_RESULTS = res

    out = np.zeros((N, D), dtype=np.float32)
    own = consts["own_nodes"]
    for c in range(C):
        oc = np.asarray(res.results[c]["outp"], dtype=np.float32)
        m = own[c] >= 0
        out[own[c][m]] = oc[m]
    out[consts["deg"] == 0] = 0.0
    return out


# revision 20
# speedup vs baseline: 1.3438x; 1.0437x over previous
"""HGT-style heterogeneous graph message passing on 8 Trainium2 cores.

v6 (1.29ms -> 457us): Pool-descriptor-minimized variant.  v2's wall was
the GpSimd (Pool) engine serially generating SWDGE descriptors at
~8ns/row for ~162k gathered rows per core (pair-table x gather +
own-node x gather + per-edge-slot table gather).  This version ships the
x rows that the table build and Q phase need ALREADY GATHERED AND
TRANSPOSED from the host (the sharding hint's "each device holds its
edge slice + gathered src/dst features"), so the only SWDGE gather left
is the irreducible per-edge-slot fetch of the on-device-computed [k'|m]
pair rows (~83k rows/core).  Each tile's gather is split in half across
SWDGE queues 0/1 whose descriptor generation runs on different Q7 core
pairs (~2x; queues 2/3 corrupt data on this runtime - do not use).
Growing chunk sizes (1,1,2,3,4,...) start the gather stream early, and
128-row-aligned (ntype,etype) groups make the table build exactly one
matmul per 128-row tile so it outpaces the 2x-rate gathers.

 - Host folds the per-(head, etype) relation transforms into per-(ntype,
   etype) 64x128 weight matrices:  a_e = <k'_src, q_dst>  with
   k' = x @ Wk[nt] @ blockdiag_h(A A^T * pri / sqrt(d)),  m = x @ Wv[nt] @
   blockdiag_h(M).
 - dst nodes are sharded across the 8 cores round-robin by degree rank, so
   all segment ops (softmax max/sum, weighted aggregation) become dense row
   reductions over degree-sorted [128, W_t] tiles.  No collectives.
 - Each core builds its deduplicated (src, etype) pair table [rows, 128] =
   [k' | m] in bf16 with dense matmuls over host-pregathered xpairT slabs
   (no on-device gather, no PE transposes), writes it to DRAM partition-major
   (contiguous HWDGE descriptors), then dma_gather's its edge slots
   (256B/row) — the one remaining Pool-engine cost.
"""

import sys

sys.path.insert(0, "/opt/trn_rl_repo")

import numpy as np
import ml_dtypes

BF16 = ml_dtypes.bfloat16

N, E = 40000, 640000
IN, H, HS = 64, 4, 16
NT, ET = 4, 8
D = H * HS  # 64
C = 8  # cores
NL = 5120  # padded local nodes per core
NTILES = NL // 128  # 40
NEG = -1.0e30

_cache = {}


def _host_prep(x, ntype, etype, src, dst):
    """Returns per-core input arrays + structural constants."""
    x = np.ascontiguousarray(np.asarray(x, dtype=np.float32))
    nt_ = np.asarray(ntype).astype(np.int64)
    et_ = np.asarray(etype).astype(np.int64)
    src = np.asarray(src).astype(np.int64)
    dst = np.asarray(dst).astype(np.int64)

    deg = np.bincount(dst, minlength=N)
    order = np.argsort(-deg, kind="stable")
    ranks = np.empty(N, dtype=np.int64)
    ranks[order] = np.arange(N)
    core_of_node = ranks % C
    local_of_node = ranks // C

    # tile widths (shared across cores): tile t covers global ranks [1024t, 1024(t+1))
    W = np.zeros(NTILES, dtype=np.int64)
    deg_by_rank = deg[order]
    for t in range(NTILES):
        lo, hi = t * 1024, min((t + 1) * 1024, N)
        W[t] = max(int(deg_by_rank[lo:hi].max()) if hi > lo else 1, 1)

    percore = []
    for c in range(C):
        ei = np.nonzero(core_of_node[dst] == c)[0]
        ld = local_of_node[dst[ei]]
        o = np.argsort(ld, kind="stable")
        percore.append((ei[o], ld[o]))

    # table chunks: small leading chunks so the first gathers start early;
    # each chunk's padded pair count must stay < 32000 (int16 gather idxs)
    CB = [0, 1, 2, 4, 7, 11, 16, 22, 28, 34, NTILES]
    NCH = len(CB) - 1
    chunk_of_tile = np.zeros(NTILES, dtype=np.int64)
    for h in range(NCH):
        chunk_of_tile[CB[h] : CB[h + 1]] = h
    cnts = np.zeros((C, NCH, NT * ET), dtype=np.int64)
    pair_data = []
    for c in range(C):
        ei, ld = percore[c]
        ch_of = chunk_of_tile[ld // 128]
        key = src[ei] * ET + et_[ei]
        chunk_pairs = []
        for h in range(NCH):
            uk = np.unique(key[ch_of == h])  # sorted keys
            g = nt_[uk // ET] * ET + (uk % ET)
            np.add.at(cnts[c, h], g, 1)
            chunk_pairs.append((uk, g))
        pair_data.append(chunk_pairs)
    # 128-row group alignment: every 128-row build tile is exactly one
    # (ntype, etype) group -> single matmul per tile, batched copies
    R = 128 * ((cnts.max(axis=0) + 127) // 128)  # [NCH, 32]
    CHRs = R.sum(axis=1)
    assert CHRs.max() < 32000, CHRs

    gbase = np.zeros((NCH, NT * ET), dtype=np.int64)
    for h in range(NCH):
        gbase[h] = np.concatenate(([0], np.cumsum(R[h])[:-1]))
    CHRs = [int(v) for v in CHRs]
    RB = np.concatenate(([0], np.cumsum(CHRs)[:-1])).astype(np.int64)
    RPtot = int(sum(CHRs))
    NS = [v // 128 for v in CHRs]  # p-major columns per chunk

    IDX8 = (8 * W).astype(np.int64)
    off8 = np.concatenate(([0], np.cumsum(IDX8)[:-1]))
    offw = np.concatenate(([0], np.cumsum(W)[:-1]))
    IDXW = int(IDX8.sum())
    ABW = int(W.sum())

    cores = []
    own_nodes = np.full((C, NL), -1, dtype=np.int64)
    for c in range(C):
        ei, ld = percore[c]
        etile = ld // 128
        key = src[ei] * ET + et_[ei]

        ownc = order[c::C]
        own_nodes[c, : len(ownc)] = ownc

        ch_of = chunk_of_tile[etile]

        rowid_of_edge = np.zeros(len(ei), dtype=np.int64)
        xp_node = np.full(RPtot, -1, dtype=np.int64)
        for h in range(NCH):
            uk, g = pair_data[c][h]  # uk sorted by key; g aligned
            po = np.argsort(g, kind="stable")
            gs = g[po]
            base_in_g = np.concatenate(
                ([0], np.cumsum(np.bincount(gs, minlength=NT * ET))[:-1])
            )
            rows_po = gbase[h][gs] + (np.arange(len(uk)) - base_in_g[gs])
            row_of_uk = np.empty(len(uk), dtype=np.int64)
            row_of_uk[po] = rows_po
            xp_node[RB[h] + row_of_uk] = uk // ET
            sel = np.nonzero(ch_of == h)[0]
            r = row_of_uk[np.searchsorted(uk, key[sel])]
            # p-major remap: DRAM row' = (r % 128) * NS_h + r // 128
            rowid_of_edge[sel] = (r % 128) * NS[h] + r // 128

        # host-pregathered, transposed x rows for the pair table build
        pn = np.where(xp_node >= 0, xp_node, 0)
        xpairT = np.ascontiguousarray(x[pn].astype(BF16).T)  # [64, RPtot]
        on = np.where(own_nodes[c] >= 0, own_nodes[c], 0)
        xownT = np.ascontiguousarray(x[on].astype(BF16).T)  # [64, NL]

        cnt = np.bincount(ld, minlength=NL)
        starts = np.concatenate(([0], np.cumsum(cnt)[:-1]))
        jpos = np.arange(len(ei)) - starts[ld]
        p_of = ld % 128

        kmidx = np.zeros((16, IDXW), dtype=np.int16)
        for t in range(NTILES):
            wt = int(W[t])
            sel = np.nonzero(etile == t)[0]
            M = np.zeros((128, wt), dtype=np.int16)
            M[p_of[sel], jpos[sel]] = rowid_of_edge[sel].astype(np.int16)
            idsl = M.T.ravel()  # list position k = j*128 + p
            kmidx[:, int(off8[t]) : int(off8[t]) + 8 * wt] = idsl.reshape(
                8 * wt, 16
            ).T

        # abias/oneh are built on device from per-node degree and type
        degf = np.ascontiguousarray(
            cnt.astype(np.float32).reshape(NTILES, 128).T
        )  # [128, NTILES]
        ntv = np.where(own_nodes[c] >= 0, nt_[on], 255).astype(np.float32)
        ntc = np.ascontiguousarray(ntv.reshape(NTILES, 128).T)  # [128, NTILES]

        cores.append(
            dict(
                xpairT=xpairT, xownT=xownT,
                degf=degf, ntc=ntc, kmidx=kmidx,
                iotaf=np.tile(np.arange(int(W.max()), dtype=np.float32), (128, 1)),
            )
        )

    consts = dict(
        W=W, WMAX=int(W.max()), NCH=NCH, CB=CB, R=R, gbase=gbase, CHRs=CHRs,
        NS=NS, RB=RB, RPtot=RPtot, IDXW=IDXW, ABW=ABW, off8=off8, offw=offw,
        own_nodes=own_nodes, deg=deg,
    )
    return cores, consts


def _fold_weights(Wk, Wq, Wv, Wa, rel_att, rel_msg, rel_pri):
    Wk = np.asarray(Wk, np.float64)
    Wq = np.asarray(Wq, np.float64)
    Wv = np.asarray(Wv, np.float64)
    Wa = np.asarray(Wa, np.float64)
    rel_att = np.asarray(rel_att, np.float64)
    rel_msg = np.asarray(rel_msg, np.float64)
    rel_pri = np.asarray(rel_pri, np.float64)
    sd = float(np.sqrt(np.float32(HS)))

    wkm = np.zeros((IN, NT * ET, 2, D), np.float64)
    for nt in range(NT):
        for et in range(ET):
            Batt = np.zeros((D, D))
            Bmsg = np.zeros((D, D))
            for h in range(H):
                A = rel_att[h, et]
                Batt[h * HS : (h + 1) * HS, h * HS : (h + 1) * HS] = (
                    A @ A.T * rel_pri[h, et] / sd
                )
                Bmsg[h * HS : (h + 1) * HS, h * HS : (h + 1) * HS] = rel_msg[h, et]
            g = nt * ET + et
            wkm[:, g, 0] = Wk[nt] @ Batt
            wkm[:, g, 1] = Wv[nt] @ Bmsg
    wkm = wkm.reshape(IN, NT * ET * 2 * D).astype(BF16)
    wkm = np.ascontiguousarray(wkm)  # [64, 4096]
    wqa = np.concatenate([Wq[t] for t in range(NT)], axis=1).astype(BF16)  # [64, 256]
    wa_all = np.concatenate([Wa[t] for t in range(NT)], axis=1).astype(BF16)
    return wkm, wqa, wa_all


def _build_program(consts):
    import concourse.mybir as mybir
    import concourse.tile as tile
    from concourse import bacc
    from concourse.masks import make_identity

    f32 = mybir.dt.float32
    bf16 = mybir.dt.bfloat16
    i16 = mybir.dt.int16
    W = consts["W"]
    WMAX = consts["WMAX"]
    NCH, CB = consts["NCH"], consts["CB"]
    R, gbase, CHRs, RB = consts["R"], consts["gbase"], consts["CHRs"], consts["RB"]
    NS = consts["NS"]
    RPtot, IDXW = consts["RPtot"], consts["IDXW"]
    CH_OF = [max(h for h in range(NCH) if CB[h] <= t) for t in range(NTILES)]
    off8 = consts["off8"]

    nc = bacc.Bacc("TRN2", target_bir_lowering=False, debug=False, num_devices=C, num_swdge_queues=4)

    xpairT = nc.dram_tensor("xpairT", [IN, RPtot], bf16, kind="ExternalInput").ap()
    xownT = nc.dram_tensor("xownT", [IN, NL], bf16, kind="ExternalInput").ap()
    wkm = nc.dram_tensor("wkm", [IN, NT * ET * 2 * D], bf16, kind="ExternalInput").ap()
    wqa = nc.dram_tensor("wqa", [IN, NT * D], bf16, kind="ExternalInput").ap()
    wa = nc.dram_tensor("wa", [D, NT * D], bf16, kind="ExternalInput").ap()
    degf = nc.dram_tensor("degf", [128, NTILES], f32, kind="ExternalInput").ap()
    ntc = nc.dram_tensor("ntc", [128, NTILES], f32, kind="ExternalInput").ap()
    iotaf = nc.dram_tensor("iotaf", [128, WMAX], f32, kind="ExternalInput").ap()
    kmidx = nc.dram_tensor("kmidx", [16, IDXW], i16, kind="ExternalInput").ap()
    outp = nc.dram_tensor("outp", [NL, D], bf16, kind="ExternalOutput").ap()
    kmtab = [
        nc.dram_tensor(f"kmtab{h}", [CHRs[h], 2 * D], bf16, kind="Internal").ap()
        for h in range(NCH)
    ]

    with tile.TileContext(nc) as tc:
        with tc.tile_pool(name="const", bufs=1) as constp, \
             tc.tile_pool(name="stage", bufs=6) as stage, \
             tc.tile_pool(name="work", bufs=3) as work, \
             tc.tile_pool(name="gtpool", bufs=5) as gtpool, \
             tc.tile_pool(name="npsum", bufs=3, space="PSUM") as npsum, \
             tc.tile_pool(name="tpsum", bufs=3, space="PSUM") as tpsum, \
             tc.tile_pool(name="mpsum", bufs=2, space="PSUM") as mpsum:

            # ---- persistent constants ----
            kmidx_s = constp.tile([128, IDXW], i16, name="kmidx_s", tag="kmidx_s")
            for k in range(8):
                nc.sync.dma_start(out=kmidx_s[16 * k : 16 * (k + 1), :], in_=kmidx[:, :])
            wkm_s = constp.tile([IN, NT * ET * 2 * D], bf16, name="wkm_s", tag="wkm_s")
            nc.sync.dma_start(out=wkm_s[:], in_=wkm[:, :])
            wqa_s = constp.tile([IN, NT * D], bf16, name="wqa_s", tag="wqa_s")
            nc.sync.dma_start(out=wqa_s[:], in_=wqa[:, :])
            wa_s = constp.tile([D, NT * D], bf16, name="wa_s", tag="wa_s")
            nc.sync.dma_start(out=wa_s[:], in_=wa[:, :])
            xown_s = constp.tile([IN, NL], bf16, name="xown_s", tag="xown_s")
            nc.sync.dma_start(out=xown_s[:], in_=xownT[:, :])
            oneh_s = constp.tile([128, NTILES * NT], f32, name="oneh_s", tag="oneh_s")
            degf_s = constp.tile([128, NTILES], f32, name="degf_s", tag="degf_s")
            nc.sync.dma_start(out=degf_s[:], in_=degf[:, :])
            ntc_s = constp.tile([128, NTILES], f32, name="ntc_s", tag="ntc_s")
            nc.sync.dma_start(out=ntc_s[:], in_=ntc[:, :])
            iotaf_s = constp.tile([128, WMAX], f32, name="iotaf_s", tag="iotaf_s")
            nc.sync.dma_start(out=iotaf_s[:], in_=iotaf[:, :])
            for t4 in range(NT):
                nc.vector.tensor_scalar(
                    out=oneh_s[:].rearrange("p (t f) -> p t f", t=NTILES)[:, :, t4],
                    in0=ntc_s[:],
                    scalar1=float(t4),
                    scalar2=None,
                    op0=mybir.AluOpType.is_equal,
                )
            abias_f = constp.tile(
                [128, NTILES * WMAX], f32, name="abias_f", tag="abias_f"
            )
            ab3 = abias_f[:].rearrange("p (t w) -> p t w", t=NTILES)
            nc.vector.tensor_tensor(
                out=ab3,
                in0=iotaf_s[:].unsqueeze(1).to_broadcast([128, NTILES, WMAX]),
                in1=degf_s[:].unsqueeze(2).to_broadcast([128, NTILES, WMAX]),
                op=mybir.AluOpType.is_lt,
            )
            nc.vector.tensor_scalar(
                out=abias_f[:],
                in0=abias_f[:],
                scalar1=1.0,
                scalar2=-NEG,
                op0=mybir.AluOpType.subtract,
                op1=mybir.AluOpType.mult,
            )
            ident = constp.tile([128, 128], bf16, name="ident", tag="ident")
            make_identity(nc, ident[:])
            qall = constp.tile([128, NTILES * D], bf16, name="qall", tag="qall")

            # ---- Q phase: typed projection of own-node x (host-pregathered) ----
            def q_phase():
                for t in range(NTILES):
                    qp = mpsum.tile([128, NT * D], f32, space="PSUM", name=f"qp{t}", tag="mp")
                    nc.tensor.matmul(
                        qp[:], lhsT=xown_s[:, t * 128 : (t + 1) * 128],
                        rhs=wqa_s[:], start=True, stop=True,
                    )
                    qtmp = work.tile([128, NT * D], f32, name=f"qtmp{t}", tag="qtmp")
                    ohb = (
                        oneh_s[:]
                        .rearrange("p (t f) -> p t f", t=NTILES)[:, t]
                        .unsqueeze(1)
                        .to_broadcast([128, D, NT])
                    )
                    nc.vector.tensor_tensor(
                        out=qtmp[:].rearrange("p (t d) -> p d t", t=NT),
                        in0=qp[:].rearrange("p (t d) -> p d t", t=NT),
                        in1=ohb,
                        op=mybir.AluOpType.mult,
                    )
                    with nc.allow_low_precision(reason="4-way one-hot select to bf16"):
                        nc.vector.tensor_reduce(
                            out=qall[:, t * D : (t + 1) * D],
                            in_=qtmp[:].rearrange("p (t d) -> p d t", t=NT),
                            axis=mybir.AxisListType.X,
                            op=mybir.AluOpType.add,
                        )

            # ---- pair-table build per chunk (dense: host-pregathered xpairT) ----
            # Groups are packed back-to-back (no 128-row alignment); each
            # 128-row output tile may span several (ntype, etype) groups, so
            # it gets one partial-M matmul per overlapped group.
            def node_chunk_emitters(h):
                # 128-aligned groups: build tile i is exactly group g_of[i]
                g_of = []
                for g in range(NT * ET):
                    g_of += [g] * (int(R[h, g]) // 128)
                n_tiles = len(g_of)
                assert n_tiles * 128 == int(CHRs[h])
                SLAB = 16
                emitters = []
                for s0 in range(0, n_tiles, SLAB):
                    emitters.append(
                        lambda s0=s0: node_slab(h, g_of, n_tiles, SLAB, s0)
                    )
                return emitters

            def node_slab(h, g_of, n_tiles, SLAB, s0):
                nb = min(SLAB, n_tiles - s0)
                row0 = s0 * 128
                rows = nb * 128
                lhs_s = stage.tile([IN, SLAB * 128], bf16, name=f"lhs_{h}_{s0}", tag="lhs")
                nc.sync.dma_start(
                    out=lhs_s[:, :rows],
                    in_=xpairT[:, int(RB[h]) + row0 : int(RB[h]) + row0 + rows],
                )
                slab = stage.tile(
                    [128, SLAB, 2 * D], bf16, name=f"slab_{h}_{s0}", tag="slab"
                )
                # p-major DRAM layout: row' = p*NS_h + a, contiguous per partition
                km3 = kmtab[h][:, :].rearrange("(p a) d -> p a d", p=128)
                for i in range(0, nb, 4):
                    nn = min(4, nb - i)
                    km_p = npsum.tile(
                        [128, 512], f32, space="PSUM", name=f"km_p{h}_{s0}_{i}", tag="km_p"
                    )
                    for j in range(nn):
                        g = g_of[s0 + i + j]
                        nc.tensor.matmul(
                            km_p[:, j * 128 : (j + 1) * 128],
                            lhsT=lhs_s[:, (i + j) * 128 : (i + j + 1) * 128],
                            rhs=wkm_s[:, g * 128 : (g + 1) * 128],
                            start=True,
                            stop=True,
                        )
                    nc.any.tensor_copy(
                        out=slab[:, i : i + nn],
                        in_=km_p[:].rearrange("p (a d) -> p a d", a=4)[:, :nn],
                    )
                    # store each 4-tile group as soon as its copy lands
                    nc.sync.dma_start(
                        out=km3[:, s0 + i : s0 + i + nn, :], in_=slab[:, i : i + nn]
                    )

            # ---- phase 3: per node-tile softmax + aggregation ----
            def p3_tile(t):
                h = CH_OF[t]
                wt = int(W[t])
                n_idx = 128 * wt
                o8 = int(off8[t])
                gt = gtpool.tile([128, WMAX, 2 * D], bf16, name=f"gt{t}", tag="gt")
                # split each tile's gather across both SWDGE queues: the two
                # Q7 core pairs generate descriptors concurrently (~2x)
                wh = wt // 2
                if wh:
                    nc.gpsimd.dma_gather(
                        out_ap=gt[:, :wh],
                        in_ap=kmtab[h][:, :],
                        idxs_ap=kmidx_s[:, o8 : o8 + 8 * wh],
                        num_idxs=128 * wh,
                        num_idxs_reg=128 * wh,
                        elem_size=2 * D,
                        single_packet=False,
                        queue_num=0,
                    )
                    nc.gpsimd.dma_gather(
                        out_ap=gt[:, wh:wt],
                        in_ap=kmtab[h][:, :],
                        idxs_ap=kmidx_s[:, o8 + 8 * wh : o8 + 8 * wt],
                        num_idxs=128 * (wt - wh),
                        num_idxs_reg=128 * (wt - wh),
                        elem_size=2 * D,
                        single_packet=False,
                        queue_num=1,
                    )
                else:
                    nc.gpsimd.dma_gather(
                        out_ap=gt[:, :wt],
                        in_ap=kmtab[h][:, :],
                        idxs_ap=kmidx_s[:, o8 : o8 + 8 * wt],
                        num_idxs=n_idx,
                        num_idxs_reg=n_idx,
                        elem_size=2 * D,
                        single_packet=False,
                        queue_num=t % 2,
                    )
                aprod = work.tile([128, WMAX, D], bf16, name=f"aprod{t}", tag="aprod")
                qb = qall[:, t * D : (t + 1) * D].unsqueeze(1).to_broadcast([128, wt, D])
                with nc.allow_low_precision(reason="bf16 attention products"):
                    nc.vector.tensor_tensor(
                        out=aprod[:, :wt], in0=gt[:, :wt, :D], in1=qb, op=mybir.AluOpType.mult
                    )
                am = work.tile([128, H, WMAX], f32, name=f"am{t}", tag="am")
                nc.vector.tensor_reduce(
                    out=am[:, :, :wt],
                    in_=aprod[:, :wt].rearrange("p w (h d) -> p h w d", h=H),
                    axis=mybir.AxisListType.X,
                    op=mybir.AluOpType.add,
                )
                amb = work.tile([128, H, WMAX], f32, name=f"amb{t}", tag="amb")
                bb = (
                    abias_f[:]
                    .rearrange("p (t w) -> p t w", t=NTILES)[:, t, :wt]
                    .unsqueeze(1)
                    .to_broadcast([128, H, wt])
                )
                nc.vector.tensor_tensor(
                    out=amb[:, :, :wt], in0=am[:, :, :wt], in1=bb, op=mybir.AluOpType.add
                )
                # softmax without max-subtraction: |a| is bounded well below
                # f32 exp overflow, and pads carry a -1e30 bias -> exp == 0.
                ex = work.tile([128, H, WMAX], bf16, name=f"ex{t}", tag="ex")
                nc.scalar.activation(
                    out=ex[:, :, :wt], in_=amb[:, :, :wt],
                    func=mybir.ActivationFunctionType.Exp,
                )
                den = work.tile([128, H], f32, name=f"den{t}", tag="den")
                nc.vector.tensor_reduce(
                    out=den[:], in_=ex[:, :, :wt],
                    axis=mybir.AxisListType.X, op=mybir.AluOpType.add,
                )
                rden = work.tile([128, H], f32, name=f"rden{t}", tag="rden")
                nc.vector.reciprocal(out=rden[:], in_=den[:])
                mprod = work.tile([128, H, HS, WMAX], bf16, name=f"mprod{t}", tag="mprod")
                mpart = gt[:, :wt, D:].rearrange("p w (h d) -> p h d w", h=H)
                ab2 = ex[:, :, :wt].unsqueeze(2).to_broadcast([128, H, HS, wt])
                nc.vector.tensor_tensor(
                    out=mprod[:, :, :, :wt], in0=mpart, in1=ab2, op=mybir.AluOpType.mult
                )
                hm = work.tile([128, D], f32, name=f"hm{t}", tag="hm")
                nc.vector.tensor_reduce(
                    out=hm[:].rearrange("p (h d) -> p h d", h=H),
                    in_=mprod[:, :, :, :wt],
                    axis=mybir.AxisListType.X,
                    op=mybir.AluOpType.add,
                )
                hm2 = work.tile([128, D], bf16, name=f"hm2{t}", tag="hm2")
                nc.vector.tensor_tensor(
                    out=hm2[:].rearrange("p (h d) -> p h d", h=H),
                    in0=hm[:].rearrange("p (h d) -> p h d", h=H),
                    in1=rden[:].unsqueeze(2).to_broadcast([128, H, HS]),
                    op=mybir.AluOpType.mult,
                )
                tp = tpsum.tile([128, 128], bf16, space="PSUM", name=f"tph{t}", tag="tp")
                nc.tensor.transpose(out=tp[:D, :], in_=hm2[:], identity=ident[:])
                hT = work.tile([D, 128], bf16, name=f"hT{t}", tag="hT")
                nc.any.tensor_copy(out=hT[:], in_=tp[:D, :])
                o4 = mpsum.tile([128, NT * D], f32, space="PSUM", name=f"o4_{t}", tag="mp")
                nc.tensor.matmul(o4[:], lhsT=hT[:], rhs=wa_s[:], start=True, stop=True)
                osel = work.tile([128, NT * D], f32, name=f"osel{t}", tag="osel")
                ohb = (
                    oneh_s[:]
                    .rearrange("p (t f) -> p t f", t=NTILES)[:, t]
                    .unsqueeze(1)
                    .to_broadcast([128, D, NT])
                )
                nc.vector.tensor_tensor(
                    out=osel[:].rearrange("p (t d) -> p d t", t=NT),
                    in0=o4[:].rearrange("p (t d) -> p d t", t=NT),
                    in1=ohb,
                    op=mybir.AluOpType.mult,
                )
                ot = work.tile([128, D], bf16, name=f"ot{t}", tag="ot")
                with nc.allow_low_precision(reason="4-way one-hot select to bf16 output"):
                    nc.vector.tensor_reduce(
                        out=ot[:],
                        in_=osel[:].rearrange("p (t d) -> p d t", t=NT),
                        axis=mybir.AxisListType.X,
                        op=mybir.AluOpType.add,
                    )
                nc.sync.dma_start(out=outp[t * 128 : (t + 1) * 128, :], in_=ot[:])

            # emission order = scheduler priority: chunk 0's table build first
            # (the first gather depends on it), then the Q phase, then
            # interleave later chunks' build slabs with phase 3 of the
            # already-built chunks.  The Pool engine only runs the p3 gathers;
            # everything else hides under its serial descriptor generation.
            for em in node_chunk_emitters(0):
                em()
            for em in node_chunk_emitters(1):
                em()
            for em in node_chunk_emitters(2):
                em()
            q_phase()
            for t in range(CB[0], CB[1]):
                p3_tile(t)
            # two-chunk build lead: while chunk h-2's tiles gather, chunk h's
            # slabs are already queued so the build never falls behind
            for h in range(3, NCH):
                slabs = node_chunk_emitters(h)
                tiles = list(range(CB[h - 2], CB[h - 1]))
                ns, ntl = len(slabs), len(tiles)
                si = ti = 0
                while si < ns or ti < ntl:
                    take = (si + 1) * ntl <= (ti + 1) * ns
                    if si < ns and (take or ti >= ntl):
                        slabs[si]()
                        si += 1
                    else:
                        p3_tile(tiles[ti])
                        ti += 1
            for t in range(CB[NCH - 2], NTILES):
                p3_tile(t)

    nc.compile()
    return nc


LAST_RESULTS = None


def kernel(x, ntype, etype, src, dst, Wk, Wq, Wv, Wa, rel_att, rel_msg, rel_pri):
    import os

    from concourse import bass_utils

    cores, consts = _host_prep(x, ntype, etype, src, dst)
    wkm, wqa, wa_all = _fold_weights(Wk, Wq, Wv, Wa, rel_att, rel_msg, rel_pri)

    struct_sig = (
        tuple(consts["W"].tolist()),
        consts["NCH"],
        tuple(consts["CHRs"]),
        tuple(consts["R"].ravel().tolist()),
    )
    if "prog" not in _cache or _cache["prog"][0] != struct_sig:
        _cache["prog"] = (struct_sig, _build_program(consts))
    nc = _cache["prog"][1]

    in_maps = [
        dict(
            xpairT=d["xpairT"], xownT=d["xownT"],
            wkm=wkm, wqa=wqa, wa=wa_all,
            degf=d["degf"], ntc=d["ntc"], iotaf=d["iotaf"], kmidx=d["kmidx"],
        )
        for c, d in enumerate(cores)
    ]
    trace = bool(int(os.environ.get("GNN_TRACE", "0")))
    res = bass_utils.run_bass_kernel_spmd(
        nc,
        in_maps,
        core_ids=list(range(C)),
        trace=trace,
        tmpdir=os.environ.get("GNN_TRACE_DIR") or None,
    )
    global LAST_RESULTS
    LAST_RESULTS = res

    out = np.zeros((N, D), dtype=np.float32)
    own = consts["own_nodes"]
    for c in range(C):
        oc = np.asarray(res.results[c]["outp"], dtype=np.float32)
        m = own[c] >= 0
        out[own[c][m]] = oc[m]
    out[consts["deg"] == 0] = 0.0
    return out


# revision 23
# speedup vs baseline: 1.5338x; 1.1413x over previous
"""HGT-style heterogeneous graph message passing on 8 Trainium2 cores.

v6 (1.29ms -> 457us): Pool-descriptor-minimized variant.  v2's wall was
the GpSimd (Pool) engine serially generating SWDGE descriptors at
~8ns/row for ~162k gathered rows per core (pair-table x gather +
own-node x gather + per-edge-slot table gather).  This version ships the
x rows that the table build and Q phase need ALREADY GATHERED AND
TRANSPOSED from the host (the sharding hint's "each device holds its
edge slice + gathered src/dst features"), so the only SWDGE gather left
is the irreducible per-edge-slot fetch of the on-device-computed [k'|m]
pair rows (~83k rows/core).  Each tile's gather is split in half across
SWDGE queues 0/1 whose descriptor generation runs on different Q7 core
pairs (~2x; queues 2/3 corrupt data on this runtime - do not use).
Growing chunk sizes (1,1,2,3,4,...) start the gather stream early, and
128-row-aligned (ntype,etype) groups make the table build exactly one
matmul per 128-row tile so it outpaces the 2x-rate gathers.

 - Host folds the per-(head, etype) relation transforms into per-(ntype,
   etype) 64x128 weight matrices:  a_e = <k'_src, q_dst>  with
   k' = x @ Wk[nt] @ blockdiag_h(A A^T * pri / sqrt(d)),  m = x @ Wv[nt] @
   blockdiag_h(M).
 - dst nodes are sharded across the 8 cores round-robin by degree rank, so
   all segment ops (softmax max/sum, weighted aggregation) become dense row
   reductions over degree-sorted [128, W_t] tiles.  No collectives.
 - Each core builds its deduplicated (src, etype) pair table [rows, 128] =
   [k' | m] in bf16 with dense matmuls over host-pregathered xpairT slabs
   (no on-device gather, no PE transposes), writes it to DRAM partition-major
   (contiguous HWDGE descriptors), then dma_gather's its edge slots
   (256B/row) — the one remaining Pool-engine cost.
"""

import sys

sys.path.insert(0, "/opt/trn_rl_repo")

import numpy as np
import ml_dtypes

BF16 = ml_dtypes.bfloat16

N, E = 40000, 640000
IN, H, HS = 64, 4, 16
NT, ET = 4, 8
D = H * HS  # 64
C = 8  # cores
NL = 5120  # padded local nodes per core
NTILES = NL // 128  # 40
NEG = -1.0e30

_cache = {}


def _host_prep(x, ntype, etype, src, dst):
    """Returns per-core input arrays + structural constants."""
    x = np.ascontiguousarray(np.asarray(x, dtype=np.float32))
    nt_ = np.asarray(ntype).astype(np.int64)
    et_ = np.asarray(etype).astype(np.int64)
    src = np.asarray(src).astype(np.int64)
    dst = np.asarray(dst).astype(np.int64)

    deg = np.bincount(dst, minlength=N)
    order = np.argsort(-deg, kind="stable")
    ranks = np.empty(N, dtype=np.int64)
    ranks[order] = np.arange(N)
    core_of_node = ranks % C
    local_of_node = ranks // C

    # tile widths (shared across cores): tile t covers global ranks [1024t, 1024(t+1))
    W = np.zeros(NTILES, dtype=np.int64)
    deg_by_rank = deg[order]
    for t in range(NTILES):
        lo, hi = t * 1024, min((t + 1) * 1024, N)
        W[t] = max(int(deg_by_rank[lo:hi].max()) if hi > lo else 1, 1)

    percore = []
    for c in range(C):
        ei = np.nonzero(core_of_node[dst] == c)[0]
        ld = local_of_node[dst[ei]]
        o = np.argsort(ld, kind="stable")
        percore.append((ei[o], ld[o]))

    # table chunks: small leading chunks so the first gathers start early;
    # each chunk's padded pair count must stay < 32000 (int16 gather idxs)
    CB = [0, 1, 2, 4, 7, 11, 16, 22, 28, 34, NTILES]
    NCH = len(CB) - 1
    chunk_of_tile = np.zeros(NTILES, dtype=np.int64)
    for h in range(NCH):
        chunk_of_tile[CB[h] : CB[h + 1]] = h
    cnts = np.zeros((C, NCH, NT * ET), dtype=np.int64)
    pair_data = []
    for c in range(C):
        ei, ld = percore[c]
        ch_of = chunk_of_tile[ld // 128]
        key = src[ei] * ET + et_[ei]
        chunk_pairs = []
        for h in range(NCH):
            uk = np.unique(key[ch_of == h])  # sorted keys
            g = nt_[uk // ET] * ET + (uk % ET)
            np.add.at(cnts[c, h], g, 1)
            chunk_pairs.append((uk, g))
        pair_data.append(chunk_pairs)
    # 128-row group alignment: every 128-row build tile is exactly one
    # (ntype, etype) group -> single matmul per tile, batched copies
    R = 128 * ((cnts.max(axis=0) + 127) // 128)  # [NCH, 32]
    CHRs = R.sum(axis=1)
    assert CHRs.max() < 32000, CHRs

    gbase = np.zeros((NCH, NT * ET), dtype=np.int64)
    for h in range(NCH):
        gbase[h] = np.concatenate(([0], np.cumsum(R[h])[:-1]))
    CHRs = [int(v) for v in CHRs]
    RB = np.concatenate(([0], np.cumsum(CHRs)[:-1])).astype(np.int64)
    RPtot = int(sum(CHRs))
    NS = [v // 128 for v in CHRs]  # p-major columns per chunk

    IDX8 = (8 * W).astype(np.int64)
    off8 = np.concatenate(([0], np.cumsum(IDX8)[:-1]))
    offw = np.concatenate(([0], np.cumsum(W)[:-1]))
    IDXW = int(IDX8.sum())
    ABW = int(W.sum())

    cores = []
    own_nodes = np.full((C, NL), -1, dtype=np.int64)
    for c in range(C):
        ei, ld = percore[c]
        etile = ld // 128
        key = src[ei] * ET + et_[ei]

        ownc = order[c::C]
        own_nodes[c, : len(ownc)] = ownc

        ch_of = chunk_of_tile[etile]

        rowid_of_edge = np.zeros(len(ei), dtype=np.int64)
        xp_node = np.full(RPtot, -1, dtype=np.int64)
        for h in range(NCH):
            uk, g = pair_data[c][h]  # uk sorted by key; g aligned
            po = np.argsort(g, kind="stable")
            gs = g[po]
            base_in_g = np.concatenate(
                ([0], np.cumsum(np.bincount(gs, minlength=NT * ET))[:-1])
            )
            rows_po = gbase[h][gs] + (np.arange(len(uk)) - base_in_g[gs])
            row_of_uk = np.empty(len(uk), dtype=np.int64)
            row_of_uk[po] = rows_po
            xp_node[RB[h] + row_of_uk] = uk // ET
            sel = np.nonzero(ch_of == h)[0]
            r = row_of_uk[np.searchsorted(uk, key[sel])]
            # p-major remap: DRAM row' = (r % 128) * NS_h + r // 128
            rowid_of_edge[sel] = (r % 128) * NS[h] + r // 128

        # host-pregathered, transposed x rows for the pair table build
        pn = np.where(xp_node >= 0, xp_node, 0)
        xpairT = np.ascontiguousarray(x[pn].astype(BF16).T)  # [64, RPtot]
        on = np.where(own_nodes[c] >= 0, own_nodes[c], 0)
        xownT = np.ascontiguousarray(x[on].astype(BF16).T)  # [64, NL]

        cnt = np.bincount(ld, minlength=NL)
        starts = np.concatenate(([0], np.cumsum(cnt)[:-1]))
        jpos = np.arange(len(ei)) - starts[ld]
        p_of = ld % 128

        kmidx = np.zeros((16, IDXW), dtype=np.int16)
        for t in range(NTILES):
            wt = int(W[t])
            sel = np.nonzero(etile == t)[0]
            M = np.zeros((128, wt), dtype=np.int16)
            M[p_of[sel], jpos[sel]] = rowid_of_edge[sel].astype(np.int16)
            idsl = M.T.ravel()  # list position k = j*128 + p
            kmidx[:, int(off8[t]) : int(off8[t]) + 8 * wt] = idsl.reshape(
                8 * wt, 16
            ).T

        # abias/oneh are built on device from per-node degree and type
        degf = np.ascontiguousarray(
            cnt.astype(np.float32).reshape(NTILES, 128).T
        )  # [128, NTILES]
        ntv = np.where(own_nodes[c] >= 0, nt_[on], 255).astype(np.float32)
        ntc = np.ascontiguousarray(ntv.reshape(NTILES, 128).T)  # [128, NTILES]

        cores.append(
            dict(
                xpairT=xpairT, xownT=xownT,
                degf=degf, ntc=ntc, kmidx=kmidx,
                iotaf=np.tile(np.arange(int(W.max()), dtype=np.float32), (128, 1)),
            )
        )

    consts = dict(
        W=W, WMAX=int(W.max()), NCH=NCH, CB=CB, R=R, gbase=gbase, CHRs=CHRs,
        NS=NS, RB=RB, RPtot=RPtot, IDXW=IDXW, ABW=ABW, off8=off8, offw=offw,
        own_nodes=own_nodes, deg=deg,
    )
    return cores, consts


def _fold_weights(Wk, Wq, Wv, Wa, rel_att, rel_msg, rel_pri):
    Wk = np.asarray(Wk, np.float64)
    Wq = np.asarray(Wq, np.float64)
    Wv = np.asarray(Wv, np.float64)
    Wa = np.asarray(Wa, np.float64)
    rel_att = np.asarray(rel_att, np.float64)
    rel_msg = np.asarray(rel_msg, np.float64)
    rel_pri = np.asarray(rel_pri, np.float64)
    sd = float(np.sqrt(np.float32(HS)))

    wkm = np.zeros((IN, NT * ET, 2, D), np.float64)
    for nt in range(NT):
        for et in range(ET):
            Batt = np.zeros((D, D))
            Bmsg = np.zeros((D, D))
            for h in range(H):
                A = rel_att[h, et]
                Batt[h * HS : (h + 1) * HS, h * HS : (h + 1) * HS] = (
                    A @ A.T * rel_pri[h, et] / sd
                )
                Bmsg[h * HS : (h + 1) * HS, h * HS : (h + 1) * HS] = rel_msg[h, et]
            g = nt * ET + et
            wkm[:, g, 0] = Wk[nt] @ Batt
            wkm[:, g, 1] = Wv[nt] @ Bmsg
    wkm = wkm.reshape(IN, NT * ET * 2 * D).astype(BF16)
    wkm = np.ascontiguousarray(wkm)  # [64, 4096]
    wqa = np.concatenate([Wq[t] for t in range(NT)], axis=1).astype(BF16)  # [64, 256]
    wa_all = np.concatenate([Wa[t] for t in range(NT)], axis=1).astype(BF16)
    return wkm, wqa, wa_all


def _build_program(consts):
    import concourse.mybir as mybir
    import concourse.tile as tile
    from concourse import bacc
    from concourse.masks import make_identity

    f32 = mybir.dt.float32
    bf16 = mybir.dt.bfloat16
    i16 = mybir.dt.int16
    W = consts["W"]
    WMAX = consts["WMAX"]
    NCH, CB = consts["NCH"], consts["CB"]
    R, gbase, CHRs, RB = consts["R"], consts["gbase"], consts["CHRs"], consts["RB"]
    NS = consts["NS"]
    RPtot, IDXW = consts["RPtot"], consts["IDXW"]
    CH_OF = [max(h for h in range(NCH) if CB[h] <= t) for t in range(NTILES)]
    off8 = consts["off8"]

    nc = bacc.Bacc("TRN2", target_bir_lowering=False, debug=False, num_devices=C, num_swdge_queues=4)

    xpairT = nc.dram_tensor("xpairT", [IN, RPtot], bf16, kind="ExternalInput").ap()
    xownT = nc.dram_tensor("xownT", [IN, NL], bf16, kind="ExternalInput").ap()
    wkm = nc.dram_tensor("wkm", [IN, NT * ET * 2 * D], bf16, kind="ExternalInput").ap()
    wqa = nc.dram_tensor("wqa", [IN, NT * D], bf16, kind="ExternalInput").ap()
    wa = nc.dram_tensor("wa", [D, NT * D], bf16, kind="ExternalInput").ap()
    degf = nc.dram_tensor("degf", [128, NTILES], f32, kind="ExternalInput").ap()
    ntc = nc.dram_tensor("ntc", [128, NTILES], f32, kind="ExternalInput").ap()
    iotaf = nc.dram_tensor("iotaf", [128, WMAX], f32, kind="ExternalInput").ap()
    kmidx = nc.dram_tensor("kmidx", [16, IDXW], i16, kind="ExternalInput").ap()
    outp = nc.dram_tensor("outp", [NL, D], bf16, kind="ExternalOutput").ap()
    kmtab = [
        nc.dram_tensor(f"kmtab{h}", [CHRs[h], 2 * D], bf16, kind="Internal").ap()
        for h in range(NCH)
    ]

    with tile.TileContext(nc) as tc:
        with tc.tile_pool(name="const", bufs=1) as constp, \
             tc.tile_pool(name="stage", bufs=6) as stage, \
             tc.tile_pool(name="work", bufs=3) as work, \
             tc.tile_pool(name="gtpool", bufs=5) as gtpool, \
             tc.tile_pool(name="npsum", bufs=3, space="PSUM") as npsum, \
             tc.tile_pool(name="tpsum", bufs=3, space="PSUM") as tpsum, \
             tc.tile_pool(name="mpsum", bufs=2, space="PSUM") as mpsum:

            # ---- persistent constants ----
            kmidx_s = constp.tile([128, IDXW], i16, name="kmidx_s", tag="kmidx_s")
            for k in range(8):
                nc.sync.dma_start(out=kmidx_s[16 * k : 16 * (k + 1), :], in_=kmidx[:, :])
            wkm_s = constp.tile([IN, NT * ET * 2 * D], bf16, name="wkm_s", tag="wkm_s")
            nc.sync.dma_start(out=wkm_s[:], in_=wkm[:, :])
            wqa_s = constp.tile([IN, NT * D], bf16, name="wqa_s", tag="wqa_s")
            nc.sync.dma_start(out=wqa_s[:], in_=wqa[:, :])
            wa_s = constp.tile([D, NT * D], bf16, name="wa_s", tag="wa_s")
            nc.sync.dma_start(out=wa_s[:], in_=wa[:, :])
            xown_s = constp.tile([IN, NL], bf16, name="xown_s", tag="xown_s")
            nc.sync.dma_start(out=xown_s[:], in_=xownT[:, :])
            oneh_s = constp.tile([128, NTILES * NT], f32, name="oneh_s", tag="oneh_s")
            degf_s = constp.tile([128, NTILES], f32, name="degf_s", tag="degf_s")
            nc.sync.dma_start(out=degf_s[:], in_=degf[:, :])
            ntc_s = constp.tile([128, NTILES], f32, name="ntc_s", tag="ntc_s")
            nc.sync.dma_start(out=ntc_s[:], in_=ntc[:, :])
            iotaf_s = constp.tile([128, WMAX], f32, name="iotaf_s", tag="iotaf_s")
            nc.sync.dma_start(out=iotaf_s[:], in_=iotaf[:, :])
            for t4 in range(NT):
                nc.vector.tensor_scalar(
                    out=oneh_s[:].rearrange("p (t f) -> p t f", t=NTILES)[:, :, t4],
                    in0=ntc_s[:],
                    scalar1=float(t4),
                    scalar2=None,
                    op0=mybir.AluOpType.is_equal,
                )
            abias_f = constp.tile(
                [128, NTILES * WMAX], f32, name="abias_f", tag="abias_f"
            )
            ab3 = abias_f[:].rearrange("p (t w) -> p t w", t=NTILES)
            nc.vector.tensor_tensor(
                out=ab3,
                in0=iotaf_s[:].unsqueeze(1).to_broadcast([128, NTILES, WMAX]),
                in1=degf_s[:].unsqueeze(2).to_broadcast([128, NTILES, WMAX]),
                op=mybir.AluOpType.is_lt,
            )
            nc.vector.tensor_scalar(
                out=abias_f[:],
                in0=abias_f[:],
                scalar1=1.0,
                scalar2=-NEG,
                op0=mybir.AluOpType.subtract,
                op1=mybir.AluOpType.mult,
            )
            ident = constp.tile([128, 128], bf16, name="ident", tag="ident")
            make_identity(nc, ident[:])
            qall = constp.tile([128, NTILES * D], bf16, name="qall", tag="qall")

            # ---- Q phase: typed projection of own-node x (host-pregathered) ----
            def q_phase():
                for t in range(NTILES):
                    qp = mpsum.tile([128, NT * D], f32, space="PSUM", name=f"qp{t}", tag="mp")
                    nc.tensor.matmul(
                        qp[:], lhsT=xown_s[:, t * 128 : (t + 1) * 128],
                        rhs=wqa_s[:], start=True, stop=True,
                    )
                    qtmp = work.tile([128, NT * D], f32, name=f"qtmp{t}", tag="qtmp")
                    ohb = (
                        oneh_s[:]
                        .rearrange("p (t f) -> p t f", t=NTILES)[:, t]
                        .unsqueeze(1)
                        .to_broadcast([128, D, NT])
                    )
                    nc.vector.tensor_tensor(
                        out=qtmp[:].rearrange("p (t d) -> p d t", t=NT),
                        in0=qp[:].rearrange("p (t d) -> p d t", t=NT),
                        in1=ohb,
                        op=mybir.AluOpType.mult,
                    )
                    with nc.allow_low_precision(reason="4-way one-hot select to bf16"):
                        nc.vector.tensor_reduce(
                            out=qall[:, t * D : (t + 1) * D],
                            in_=qtmp[:].rearrange("p (t d) -> p d t", t=NT),
                            axis=mybir.AxisListType.X,
                            op=mybir.AluOpType.add,
                        )

            # ---- pair-table build per chunk (dense: host-pregathered xpairT) ----
            # Groups are packed back-to-back (no 128-row alignment); each
            # 128-row output tile may span several (ntype, etype) groups, so
            # it gets one partial-M matmul per overlapped group.
            def node_chunk_emitters(h):
                # 128-aligned groups: build tile i is exactly group g_of[i]
                g_of = []
                for g in range(NT * ET):
                    g_of += [g] * (int(R[h, g]) // 128)
                n_tiles = len(g_of)
                assert n_tiles * 128 == int(CHRs[h])
                SLAB = 16
                emitters = []
                for s0 in range(0, n_tiles, SLAB):
                    emitters.append(
                        lambda s0=s0: node_slab(h, g_of, n_tiles, SLAB, s0)
                    )
                return emitters

            def node_slab(h, g_of, n_tiles, SLAB, s0):
                nb = min(SLAB, n_tiles - s0)
                row0 = s0 * 128
                rows = nb * 128
                lhs_s = stage.tile([IN, SLAB * 128], bf16, name=f"lhs_{h}_{s0}", tag="lhs")
                nc.sync.dma_start(
                    out=lhs_s[:, :rows],
                    in_=xpairT[:, int(RB[h]) + row0 : int(RB[h]) + row0 + rows],
                )
                slab = stage.tile(
                    [128, SLAB, 2 * D], bf16, name=f"slab_{h}_{s0}", tag="slab"
                )
                # p-major DRAM layout: row' = p*NS_h + a, contiguous per partition
                km3 = kmtab[h][:, :].rearrange("(p a) d -> p a d", p=128)
                for i in range(0, nb, 4):
                    nn = min(4, nb - i)
                    km_p = npsum.tile(
                        [128, 512], f32, space="PSUM", name=f"km_p{h}_{s0}_{i}", tag="km_p"
                    )
                    for j in range(nn):
                        g = g_of[s0 + i + j]
                        nc.tensor.matmul(
                            km_p[:, j * 128 : (j + 1) * 128],
                            lhsT=lhs_s[:, (i + j) * 128 : (i + j + 1) * 128],
                            rhs=wkm_s[:, g * 128 : (g + 1) * 128],
                            start=True,
                            stop=True,
                        )
                    nc.any.tensor_copy(
                        out=slab[:, i : i + nn],
                        in_=km_p[:].rearrange("p (a d) -> p a d", a=4)[:, :nn],
                    )
                    # store each 4-tile group as soon as its copy lands
                    nc.sync.dma_start(
                        out=km3[:, s0 + i : s0 + i + nn, :], in_=slab[:, i : i + nn]
                    )

            # ---- phase 3: per node-tile softmax + aggregation ----
            def p3_tile(t):
                h = CH_OF[t]
                wt = int(W[t])
                n_idx = 128 * wt
                o8 = int(off8[t])
                gt = gtpool.tile([128, WMAX, 2 * D], bf16, name=f"gt{t}", tag="gt")
                # split each tile's gather across SWDGE queues: each queue's
                # descriptors are generated by its own Q7 core pair (~Nx)
                wh = wt // 2
                if wt >= 3:
                    w3 = [0, wt // 3, (2 * wt) // 3, wt]
                    for k in range(3):
                        a, b = w3[k], w3[k + 1]
                        nc.gpsimd.dma_gather(
                            out_ap=gt[:, a:b],
                            in_ap=kmtab[h][:, :],
                            idxs_ap=kmidx_s[:, o8 + 8 * a : o8 + 8 * b],
                            num_idxs=128 * (b - a),
                            num_idxs_reg=128 * (b - a),
                            elem_size=2 * D,
                            single_packet=False,
                            queue_num=k,
                        )
                elif wt == 2:
                    nc.gpsimd.dma_gather(
                        out_ap=gt[:, :wh],
                        in_ap=kmtab[h][:, :],
                        idxs_ap=kmidx_s[:, o8 : o8 + 8 * wh],
                        num_idxs=128 * wh,
                        num_idxs_reg=128 * wh,
                        elem_size=2 * D,
                        single_packet=False,
                        queue_num=0,
                    )
                    nc.gpsimd.dma_gather(
                        out_ap=gt[:, wh:wt],
                        in_ap=kmtab[h][:, :],
                        idxs_ap=kmidx_s[:, o8 + 8 * wh : o8 + 8 * wt],
                        num_idxs=128 * (wt - wh),
                        num_idxs_reg=128 * (wt - wh),
                        elem_size=2 * D,
                        single_packet=False,
                        queue_num=1,
                    )
                else:
                    nc.gpsimd.dma_gather(
                        out_ap=gt[:, :wt],
                        in_ap=kmtab[h][:, :],
                        idxs_ap=kmidx_s[:, o8 : o8 + 8 * wt],
                        num_idxs=n_idx,
                        num_idxs_reg=n_idx,
                        elem_size=2 * D,
                        single_packet=False,
                        queue_num=t % 2,
                    )
                aprod = work.tile([128, WMAX, D], bf16, name=f"aprod{t}", tag="aprod")
                qb = qall[:, t * D : (t + 1) * D].unsqueeze(1).to_broadcast([128, wt, D])
                with nc.allow_low_precision(reason="bf16 attention products"):
                    nc.vector.tensor_tensor(
                        out=aprod[:, :wt], in0=gt[:, :wt, :D], in1=qb, op=mybir.AluOpType.mult
                    )
                am = work.tile([128, H, WMAX], f32, name=f"am{t}", tag="am")
                nc.vector.tensor_reduce(
                    out=am[:, :, :wt],
                    in_=aprod[:, :wt].rearrange("p w (h d) -> p h w d", h=H),
                    axis=mybir.AxisListType.X,
                    op=mybir.AluOpType.add,
                )
                amb = work.tile([128, H, WMAX], f32, name=f"amb{t}", tag="amb")
                bb = (
                    abias_f[:]
                    .rearrange("p (t w) -> p t w", t=NTILES)[:, t, :wt]
                    .unsqueeze(1)
                    .to_broadcast([128, H, wt])
                )
                nc.vector.tensor_tensor(
                    out=amb[:, :, :wt], in0=am[:, :, :wt], in1=bb, op=mybir.AluOpType.add
                )
                # softmax without max-subtraction: |a| is bounded well below
                # f32 exp overflow, and pads carry a -1e30 bias -> exp == 0.
                ex = work.tile([128, H, WMAX], bf16, name=f"ex{t}", tag="ex")
                nc.scalar.activation(
                    out=ex[:, :, :wt], in_=amb[:, :, :wt],
                    func=mybir.ActivationFunctionType.Exp,
                )
                den = work.tile([128, H], f32, name=f"den{t}", tag="den")
                nc.vector.tensor_reduce(
                    out=den[:], in_=ex[:, :, :wt],
                    axis=mybir.AxisListType.X, op=mybir.AluOpType.add,
                )
                rden = work.tile([128, H], f32, name=f"rden{t}", tag="rden")
                nc.vector.reciprocal(out=rden[:], in_=den[:])
                mprod = work.tile([128, H, HS, WMAX], bf16, name=f"mprod{t}", tag="mprod")
                mpart = gt[:, :wt, D:].rearrange("p w (h d) -> p h d w", h=H)
                ab2 = ex[:, :, :wt].unsqueeze(2).to_broadcast([128, H, HS, wt])
                nc.vector.tensor_tensor(
                    out=mprod[:, :, :, :wt], in0=mpart, in1=ab2, op=mybir.AluOpType.mult
                )
                hm = work.tile([128, D], f32, name=f"hm{t}", tag="hm")
                nc.vector.tensor_reduce(
                    out=hm[:].rearrange("p (h d) -> p h d", h=H),
                    in_=mprod[:, :, :, :wt],
                    axis=mybir.AxisListType.X,
                    op=mybir.AluOpType.add,
                )
                hm2 = work.tile([128, D], bf16, name=f"hm2{t}", tag="hm2")
                nc.vector.tensor_tensor(
                    out=hm2[:].rearrange("p (h d) -> p h d", h=H),
                    in0=hm[:].rearrange("p (h d) -> p h d", h=H),
                    in1=rden[:].unsqueeze(2).to_broadcast([128, H, HS]),
                    op=mybir.AluOpType.mult,
                )
                tp = tpsum.tile([128, 128], bf16, space="PSUM", name=f"tph{t}", tag="tp")
                nc.tensor.transpose(out=tp[:D, :], in_=hm2[:], identity=ident[:])
                hT = work.tile([D, 128], bf16, name=f"hT{t}", tag="hT")
                nc.any.tensor_copy(out=hT[:], in_=tp[:D, :])
                o4 = mpsum.tile([128, NT * D], f32, space="PSUM", name=f"o4_{t}", tag="mp")
                nc.tensor.matmul(o4[:], lhsT=hT[:], rhs=wa_s[:], start=True, stop=True)
                osel = work.tile([128, NT * D], f32, name=f"osel{t}", tag="osel")
                ohb = (
                    oneh_s[:]
                    .rearrange("p (t f) -> p t f", t=NTILES)[:, t]
                    .unsqueeze(1)
                    .to_broadcast([128, D, NT])
                )
                nc.vector.tensor_tensor(
                    out=osel[:].rearrange("p (t d) -> p d t", t=NT),
                    in0=o4[:].rearrange("p (t d) -> p d t", t=NT),
                    in1=ohb,
                    op=mybir.AluOpType.mult,
                )
                ot = work.tile([128, D], bf16, name=f"ot{t}", tag="ot")
                with nc.allow_low_precision(reason="4-way one-hot select to bf16 output"):
                    nc.vector.tensor_reduce(
                        out=ot[:],
                        in_=osel[:].rearrange("p (t d) -> p d t", t=NT),
                        axis=mybir.AxisListType.X,
                        op=mybir.AluOpType.add,
                    )
                nc.sync.dma_start(out=outp[t * 128 : (t + 1) * 128, :], in_=ot[:])

            # emission order = scheduler priority: chunk 0's table build first
            # (the first gather depends on it), then the Q phase, then
            # interleave later chunks' build slabs with phase 3 of the
            # already-built chunks.  The Pool engine only runs the p3 gathers;
            # everything else hides under its serial descriptor generation.
            for em in node_chunk_emitters(0):
                em()
            for em in node_chunk_emitters(1):
                em()
            for em in node_chunk_emitters(2):
                em()
            q_phase()
            for t in range(CB[0], CB[1]):
                p3_tile(t)
            # two-chunk build lead: while chunk h-2's tiles gather, chunk h's
            # slabs are already queued so the build never falls behind
            for h in range(3, NCH):
                slabs = node_chunk_emitters(h)
                tiles = list(range(CB[h - 2], CB[h - 1]))
                ns, ntl = len(slabs), len(tiles)
                si = ti = 0
                while si < ns or ti < ntl:
                    take = (si + 1) * ntl <= (ti + 1) * ns
                    if si < ns and (take or ti >= ntl):
                        slabs[si]()
                        si += 1
                    else:
                        p3_tile(tiles[ti])
                        ti += 1
            for t in range(CB[NCH - 2], NTILES):
                p3_tile(t)

    nc.compile()
    return nc


LAST_RESULTS = None


def kernel(x, ntype, etype, src, dst, Wk, Wq, Wv, Wa, rel_att, rel_msg, rel_pri):
    import os

    from concourse import bass_utils

    cores, consts = _host_prep(x, ntype, etype, src, dst)
    wkm, wqa, wa_all = _fold_weights(Wk, Wq, Wv, Wa, rel_att, rel_msg, rel_pri)

    struct_sig = (
        tuple(consts["W"].tolist()),
        consts["NCH"],
        tuple(consts["CHRs"]),
        tuple(consts["R"].ravel().tolist()),
    )
    if "prog" not in _cache or _cache["prog"][0] != struct_sig:
        _cache["prog"] = (struct_sig, _build_program(consts))
    nc = _cache["prog"][1]

    in_maps = [
        dict(
            xpairT=d["xpairT"], xownT=d["xownT"],
            wkm=wkm, wqa=wqa, wa=wa_all,
            degf=d["degf"], ntc=d["ntc"], iotaf=d["iotaf"], kmidx=d["kmidx"],
        )
        for c, d in enumerate(cores)
    ]
    trace = bool(int(os.environ.get("GNN_TRACE", "0")))
    res = bass_utils.run_bass_kernel_spmd(
        nc,
        in_maps,
        core_ids=list(range(C)),
        trace=trace,
        tmpdir=os.environ.get("GNN_TRACE_DIR") or None,
    )
    global LAST_RESULTS
    LAST_RESULTS = res

    out = np.zeros((N, D), dtype=np.float32)
    own = consts["own_nodes"]
    for c in range(C):
        oc = np.asarray(res.results[c]["outp"], dtype=np.float32)
        m = own[c] >= 0
        out[own[c][m]] = oc[m]
    out[consts["deg"] == 0] = 0.0
    return out
